# revision 53
# baseline (speedup 1.0000x reference)
"""Trainium2 Bass kernel for an ODE-RNN encoder (z0 posterior).

Model: 128-step reversed-time GRU-like recurrence with an Euler ODE step on
the mean channel, then a final transform producing (mean_z0, std_z0).

Strategy: data-parallel over the subject (batch) dim across 8 NeuronCores,
weights replicated.  Everything runs on-chip in a transposed layout
([feature, batch], batch=256 on the free dim).  Key points vs a naive port:
- matmul operands and the recurrent state are bf16 (fp32 PSUM accumulate).
- biases ride a ones-row appended to the streamed x tile (layer-1 nets),
  ACT per-partition bias vectors, or K=1 matmuls; zero per-step bias ops.
- the Euler step is folded into the ode2 weights: dt takes very few distinct
  values over the scan, so dt*ode_w2 / dt*ode_b2 are pre-baked per distinct
  value and Yode = Ym + psum(ode2_dt) is a single DVE add (no ACT hop).
- the reset-gate application r.Y = 0.5(1+tanh(zR/2)).Y uses 0.5-pre-scaled
  ns1 weights and am2 = (1+T).Yode as one scalar_tensor_tensor op, so ns1
  costs only K=66 (x) + 2x K=128 (state) matmuls.
- the observation mask m is computed on the host, packed as a 0.5*m row in
  the x stream, and broadcast across partitions with a K=1 matmul; the gate
  factor  -G = (tanh(zU/2) - 1)*(0.5 m)  is one STT op and the blends use
  `subtract` to absorb the sign.
- sigmoid(z) = 0.5 + 0.5*tanh(z/2) keeps every transcendental in the
  resident `exp_and_others` ACT table set (no per-step table switches).
- softplus(x) = log1p(exp(x)) via one Newton step: with w = exp(-|x|) and
  seed y0 = relu(x) + ln2*w, the correction (1+e^x)e^{-y0} collapses
  EXACTLY to (1+w)*2^{-w}, so the tail is 3 ACT ops (Abs, Exp, Exp) and a
  short DVE chain (~1.8e-3 max abs err, under the bf16 noise floor).
- TRN2 allows ONE sync wait per instruction; Bacc legalizes the rest, but
  K=1 dummy matmuls + accumulation-group ordering keep the PE free of
  multi-wait event-semaphore preambles in the steady state.
"""
import sys
import numpy as np
import ml_dtypes

for _p in ("/opt/trn_rl_repo", "/root/.axon_site/_ro/trn_rl_repo"):
    if _p not in sys.path:
        sys.path.append(_p)

N_SUBJ, N_TP, INPUT_DIM, LATENT, N_UNIT = 2048, 128, 64, 128, 256
HALF = INPUT_DIM // 2
N_CORES = 8
B = N_SUBJ // N_CORES          # 256 subjects per core (free dim)
L = LATENT
SP_ITERS = 1                   # kept for test.py compat (cache key)
LN2 = float(np.log(2.0))
BF = ml_dtypes.bfloat16

_CACHE = {}


# --------------------------------------------------------------------------
# Bass program
# --------------------------------------------------------------------------
def _build(n_tp, sp_iters, vids):
    """vids: per-step index into the distinct-dt weight variants."""
    import concourse.mybir as mybir
    from concourse import bacc, tile

    F32 = mybir.dt.float32
    B16 = mybir.dt.bfloat16
    FP8 = mybir.dt.float8e4
    DR = mybir.MatmulPerfMode.DoubleRow
    AF = mybir.ActivationFunctionType
    OP = mybir.AluOpType

    n_var = max(vids) + 1

    # Bacc (not plain Bass): its compile() legalizes the TRN2 one-sync-wait-
    # per-instruction limit (event-semaphore splitting, matmul-wait moves).
    nc = bacc.Bacc(None)

    # ---- DRAM I/O ----
    # x slab per step: row 0 = 0.5*mask-observed, rows 1..64 data
    d_x = nc.dram_tensor("x_rev", [n_tp, INPUT_DIM + 1, B], B16,
                         kind="ExternalInput")

    bspec = {  # bf16 weights (matmul operands)
        "ug1_k0": [L, N_UNIT], "ug1_k1": [L, N_UNIT], "ug1_kx": [INPUT_DIM + 2, N_UNIT],
        "rg1_k0": [L, N_UNIT], "rg1_k1": [L, N_UNIT], "rg1_kx": [INPUT_DIM + 2, N_UNIT],
        "ns1_k0": [L, N_UNIT], "ns1_k1": [L, N_UNIT], "ns1_kx": [INPUT_DIM + 2, N_UNIT],
        "ode1_w": [L, N_UNIT],
        "ns2_k0": [128, 2 * L], "ns2_k1": [128, 2 * L], "ns2_bm16": [1, L],
        "neg_eye": [L, L],
        "tz1_k0": [L, N_UNIT], "tz1_k1": [L, N_UNIT], "tz1_b": [1, N_UNIT],
        "tz2_k0": [128, 2 * L], "tz2_k1": [128, 2 * L],
    }
    # fp8 DoubleRow weights: K=256 reductions in one PE instruction (2 rows
    # per cycle).  Host-validated: fp8 on the gate layer-2 and the dt-baked
    # ode layer-2 keeps end-to-end error ~9e-3 (budget 2e-2); ns1/ns2 in fp8
    # would blow it.
    f8spec = {"ug2_k01": [128, 2, L], "rg2_k01": [128, 2, L]}
    for u in range(n_var):  # dt-baked ode layer-2 weights + dt*b2 rows
        f8spec[f"o2k01_{u}"] = [128, 2, L]
        bspec[f"o2b_{u}"] = [1, L]
    # ode layer-1 bias halves as K=1 matmul rows (they open the psB bank so
    # the batched h_ode tanh needs no per-partition ACT bias)
    bspec["o1br0"] = [1, 128]
    bspec["o1br1"] = [1, 128]
    fspec = {  # fp32 per-partition columns (ACT bias vectors)
        "ug2_bc": [128, 1], "rg2_bc": [128, 1],
        "ns2_bs": [128, 1], "tz2_bm": [128, 1], "tz2_bs": [128, 1],
    }
    d_w = {k: nc.dram_tensor(k, v, B16, kind="ExternalInput") for k, v in bspec.items()}
    d_w.update({k: nc.dram_tensor(k, v, FP8, kind="ExternalInput")
                for k, v in f8spec.items()})
    d_w.update({k: nc.dram_tensor(k, v, F32, kind="ExternalInput")
                for k, v in fspec.items()})

    d_om = nc.dram_tensor("out_m", [L, B], F32, kind="ExternalOutput")
    d_os = nc.dram_tensor("out_s", [L, B], F32, kind="ExternalOutput")

    with tile.TileContext(nc) as tc:
        with (
            tc.tile_pool(name="const", bufs=1) as cp,
            tc.tile_pool(name="work", bufs=3) as wp,
            tc.tile_pool(name="ps", bufs=1, space="PSUM") as pp,
        ):
            # ---- resident constants / weights ----
            w = {}
            for k, shp in bspec.items():
                w[k] = cp.tile(shp, B16, tag=k, name=k)
                nc.sync.dma_start(w[k][:], d_w[k][:])
            for k, shp in f8spec.items():
                w[k] = cp.tile(shp, FP8, tag=k, name=k)
                nc.sync.dma_start(w[k][:], d_w[k][:])
            for k, shp in fspec.items():
                w[k] = cp.tile(shp, F32, tag=k, name=k)
                nc.sync.dma_start(w[k][:], d_w[k][:])
            ones_row = cp.tile([1, B], B16, tag="ones_row", name="ones_row")
            nc.vector.memset(ones_row[:], 1.0)
            ones1 = cp.tile([1, 128], B16, tag="ones1", name="ones1")
            nc.vector.memset(ones1[:], 1.0)

            xbufs = []
            for j in range(3):
                xb = cp.tile([INPUT_DIM + 2, B], B16, tag=f"xb{j}", name=f"xb{j}")
                # rows 0..64 are DMA-overwritten each step; row 65 stays 1.0
                # (memset must start at a 32-aligned partition, so cover all)
                nc.vector.memset(xb[:], 1.0)
                xbufs.append(xb)

            # state lives in bf16 (matmul-input rounding dominates anyway)
            ym = [cp.tile([L, B], B16, tag=f"ym{i}", name=f"ym{i}") for i in range(2)]
            ys = [cp.tile([L, B], B16, tag=f"ys{i}", name=f"ys{i}") for i in range(2)]
            nc.vector.memset(ym[0][:], 0.0)
            nc.vector.memset(ys[0][:], 0.0)

            mm = nc.tensor.matmul

            # Warm the PE's clock past every weight DMA with K=1 dummy
            # matmuls so steady-state matmuls only wait on one producer.
            scr = pp.tile([1, 16], F32, tag="scr", name="scr")
            for k in bspec:
                mm(scr[0:1, 0:1], w[k][0:1, 0:1], w[k][0:1, 1:2],
                   start=True, stop=True)
            for k in f8spec:
                mm(scr[0:1, 0:1], w[k][0:1, 0:1, 0:1], w[k][0:1, 0:1, 1:2],
                   start=True, stop=True)
            # DVE/ACT read fp32 DMA-produced columns: warm those clocks too
            nf = len(fspec)
            warm_dv = cp.tile([1, 2 * nf], F32, tag="warm_dv", name="warm_dv")
            for j, k in enumerate(fspec):
                nc.vector.tensor_copy(warm_dv[0:1, j:j + 1], w[k][0:1, 0:1])
                nc.scalar.copy(warm_dv[0:1, nf + j:nf + j + 1], w[k][0:1, 0:1])

            # first x slab
            nc.sync.dma_start(xbufs[0][:INPUT_DIM + 1, :], d_x[0])

            # ---- the recurrence ----
            from concourse.tile_rust import add_dep_helper
            cc = float(np.float32(1e-6) - np.float32(1.0))
            for t in range(n_tp):
                cur, nxt = t % 2, (t + 1) % 2
                xb = xbufs[t % 3]
                u = vids[t]
                if t + 1 < n_tp:  # prefetch next step's x slab
                    nc.sync.dma_start(xbufs[(t + 1) % 3][:INPUT_DIM + 1, :],
                                      d_x[t + 1])
                # absorb this step's x-DMA wait into a K=1 dummy
                mm(scr[0:1, 0:1], xb[0:1, 0:1], xb[0:1, 1:2], start=True, stop=True)

                # One start=True per PSUM bank per step (it clears the whole
                # bank's has_written bits); every other matmul accumulates or
                # first-touch-overwrites per element, so groups can interleave
                # freely.  x-only matmuls go first: they are ready before the
                # previous step's state tail finishes, keeping the PE fed.
                psA = pp.tile([128, 4 * B], F32, tag="psA", name="psA")
                psC = pp.tile([128, 2 * B], F32, tag="psC", name="psC")
                psF = pp.tile([128, 2 * B], F32, tag="psF", name="psF")
                # dt*ode_b2 broadcast opens the psF bank (always-ready K=1)
                mm(psF[:, 0:B], w[f"o2b_{u}"][:], ones_row[:],
                   start=True, stop=False)
                # host-computed 0.5*mask row broadcast to all partitions
                mm(psF[:, B:], ones1[:], xb[0:1, :],
                   start=False, stop=False)
                for gi, net in ((1, "rg1"), (0, "ug1")):
                    for m in range(2):
                        sl = psA[:, (2 * gi + m) * B:(2 * gi + m + 1) * B]
                        ms = slice(m * 128, (m + 1) * 128)
                        mm(sl, w[net + "_kx"][:, ms], xb[:],
                           start=(m == 0), stop=False)
                for m in range(2):
                    ms = slice(m * 128, (m + 1) * 128)
                    mm(psC[:, m * B:(m + 1) * B], w["ns1_kx"][:, ms], xb[:],
                       start=(m == 0), stop=False)

                # ODE hidden: tanh(ode_w1^T @ Ym + b1), fp8 out; the whole
                # dt-baked K=256 ode layer-2 is ONE fp8 DoubleRow matmul, so
                # the tanh is one batched ACT op (the bias rides the psB bank
                # via always-ready K=1 matmuls that also open the bank early)
                psB = pp.tile([128, 2 * B], F32, tag="psB", name="psB")
                h_ode = wp.tile([128, 2, B], FP8, tag="h_ode", name="h_ode")
                i_b1r = mm(psB[:, 0:B], w["o1br0"][:], ones_row[:],
                           start=True, stop=False)
                mm(psB[:, B:], w["o1br1"][:], ones_row[:],
                   start=False, stop=False)
                for m in range(2):
                    sl = psB[:, m * B:(m + 1) * B]
                    ms = slice(m * 128, (m + 1) * 128)
                    i_od = mm(sl, w["ode1_w"][:, ms], ym[cur][:],
                              start=False, stop=(m == 1))
                    if m == 0:
                        add_dep_helper(i_od.ins, i_b1r.ins, False,
                                       "bank-start order")
                nc.scalar.activation(h_ode[:], psB[:], AF.Tanh)
                mm(psF[:, 0:B], w[f"o2k01_{u}"][:], h_ode[:],
                   start=False, stop=True, perf_mode=DR)

                # Yode = Ym + dt*(ode_out + b2), dt baked into o2k*/o2b:
                # a single DVE add off the PSUM accumulator
                yode = wp.tile([L, B], B16, tag="yode", name="yode")
                nc.vector.tensor_tensor(yode[:], psF[:, 0:B], ym[cur][:], op=OP.add)

                # gate layer 1 remaining k-tiles; yode (ready first) then ys,
                # per-group contiguous so each m-half completes as soon as its
                # last input lands and its tanh can fire.  The rg m0 pair is
                # pinned to run first: its tanh is on the critical std cycle,
                # and the scheduler otherwise queues rg_k1m0 behind ug work
                # (~1.7us of cycle per step).
                i_g1 = {}
                for gi, net in ((1, "rg1"), (0, "ug1")):
                    for m in range(2):
                        sl = psA[:, (2 * gi + m) * B:(2 * gi + m + 1) * B]
                        ms = slice(m * 128, (m + 1) * 128)
                        i_g1[net, m, 0] = mm(sl, w[net + "_k0"][:, ms], yode[:],
                                             start=False, stop=False)
                        i_g1[net, m, 1] = mm(sl, w[net + "_k1"][:, ms], ys[cur][:],
                                             start=False, stop=(m == 1))


                # layer 2 per gate; rg (reset gate) first: the critical
                # chain runs through R -> as2 -> ns1, U is only needed at
                # the final blend.  rg hidden tanh split per half for the
                # chain; ug hidden batched into one ACT op (off-chain).
                h_g1 = wp.tile([128, 4, B], FP8, tag="h_g1", name="h_g1")
                psD = pp.tile([128, 2 * B], F32, tag="psD", name="psD")
                t_ur = wp.tile([128, 2 * B], B16, tag="t_ur", name="t_ur")
                # one batched tanh: the DoubleRow rg2 matmul consumes both
                # halves at once, so splitting buys no early start and the
                # batch saves ~270ns of serial ACT on the cycle
                nc.scalar.activation(h_g1[:, 2:4, :], psA[:, 2 * B:4 * B],
                                     AF.Tanh)
                mm(psD[:, B:], w["rg2_k01"][:], h_g1[:, 2:4, :],
                   start=True, stop=False, perf_mode=DR)
                i_tur_r = nc.scalar.activation(t_ur[:, B:], psD[:, B:], AF.Tanh,
                                               bias=w["rg2_bc"][:, 0:1], scale=0.5)
                i_ugh = nc.scalar.activation(h_g1[:, 0:2, :], psA[:, 0:2 * B],
                                             AF.Tanh)
                add_dep_helper(i_ugh.ins, i_tur_r.ins, False, "rg ACT priority")
                mm(psD[:, 0:B], w["ug2_k01"][:], h_g1[:, 0:2, :],
                   start=False, stop=True, perf_mode=DR)
                i_tur_u = nc.scalar.activation(t_ur[:, 0:B], psD[:, 0:B], AF.Tanh,
                                               bias=w["ug2_bc"][:, 0:1], scale=0.5)

                # reset-gate products (ns1 k0/k1 pre-scaled 0.5, so
                # r.Y = 0.5(1+T).Y needs only (1+T).Y here); as2 first: the
                # std channel is the critical cycle
                as2 = wp.tile([L, B], B16, tag="as2", name="as2")
                nc.vector.scalar_tensor_tensor(
                    as2[:], t_ur[:, B:], 1.0, ys[cur][:], op0=OP.add, op1=OP.mult)
                am2 = wp.tile([L, B], B16, tag="am2", name="am2")
                nc.vector.scalar_tensor_tensor(
                    am2[:], t_ur[:, B:], 1.0, yode[:], op0=OP.add, op1=OP.mult)
                for m in range(2):
                    sl = psC[:, m * B:(m + 1) * B]
                    ms = slice(m * 128, (m + 1) * 128)
                    mm(sl, w["ns1_k1"][:, ms], as2[:], start=False, stop=False)
                    mm(sl, w["ns1_k0"][:, ms], am2[:], start=False, stop=(m == 1))

                # new-state layer 2: NM | NS pre-acts.  The NM half also
                # accumulates (+bm - Yode); nosync deps keep the bank's
                # start=True matmul first in the PE schedule.
                h_ns = wp.tile([128, 2 * B], B16, tag="h_ns", name="h_ns")
                psE = pp.tile([128, 2 * B], F32, tag="psE", name="psE")
                # bm (always ready) opens the bank and ne (yode-gated) joins
                # it early in the middle, so only the four h_ns-gated matmuls
                # remain between the tanh and the psE group close that
                # releases the tail's readers
                i_bm = mm(psE[:, 0:B], w["ns2_bm16"][:], ones_row[:],
                          start=True, stop=False)
                i_ne = mm(psE[:, 0:B], w["neg_eye"][:], yode[:],
                          start=False, stop=False)
                add_dep_helper(i_ne.ins, i_bm.ins, False, "bank-start order")
                nc.scalar.activation(h_ns[:], psC[:], AF.Tanh)
                i_k0s = mm(psE[:, B:], w["ns2_k0"][:, 128:], h_ns[:, 0:B],
                           start=False, stop=False)
                add_dep_helper(i_k0s.ins, i_ne.ins, False, "bank-start order")
                mm(psE[:, 0:B], w["ns2_k0"][:, 0:128], h_ns[:, 0:B],
                   start=False, stop=False)
                mm(psE[:, B:], w["ns2_k1"][:, 128:], h_ns[:, B:],
                   start=False, stop=False)
                mm(psE[:, 0:B], w["ns2_k1"][:, 0:128], h_ns[:, B:],
                   start=False, stop=True)

                # -G = (tanh(zU/2) - 1) * 0.5m   (one STT; sign absorbed by
                # `subtract` in the blends)
                g = wp.tile([L, B], F32, tag="g", name="g")
                nc.vector.scalar_tensor_tensor(
                    g[:], t_ur[:, 0:B], 1.0, psF[:, B:], op0=OP.subtract,
                    op1=OP.mult)

                # std tail: softplus(x)=log1p(e^x) via one Newton step.
                # w = exp(-|x|); sp = relu(x) + ln2*w - 1 + (1+w)*2^{-w}
                rl = wp.tile([L, B], F32, tag="rl", name="rl")
                nc.vector.tensor_scalar(rl[:], psE[:, B:], w["ns2_bs"][:, 0:1],
                                        0.0, op0=OP.add, op1=OP.max)

                # mean channel: Ym' = Yode - (-G)*(NM + bm - Yode)
                pm = wp.tile([L, B], F32, tag="pm", name="pm")
                nc.vector.tensor_tensor(pm[:], g[:], psE[:, 0:B], op=OP.mult)
                nc.vector.tensor_tensor(ym[nxt][:], yode[:], pm[:], op=OP.subtract)

                xa = wp.tile([L, B], F32, tag="xa", name="xa")
                nc.scalar.activation(xa[:], psE[:, B:], AF.Abs,
                                     bias=w["ns2_bs"][:, 0:1])
                wx = wp.tile([L, B], F32, tag="wx", name="wx")
                nc.scalar.activation(wx[:], xa[:], AF.Exp, scale=-1.0)
                vx = wp.tile([L, B], F32, tag="vx", name="vx")
                nc.scalar.activation(vx[:], wx[:], AF.Exp, scale=-LN2)
                h0 = wp.tile([L, B], F32, tag="h0", name="h0")
                nc.vector.scalar_tensor_tensor(
                    h0[:], wx[:], LN2, rl[:], op0=OP.mult, op1=OP.add)
                h1 = wp.tile([L, B], F32, tag="h1", name="h1")
                nc.vector.scalar_tensor_tensor(
                    h1[:], h0[:], cc, ys[cur][:], op0=OP.add, op1=OP.subtract)
                aw = wp.tile([L, B], F32, tag="aw", name="aw")
                nc.vector.scalar_tensor_tensor(
                    aw[:], wx[:], 1.0, vx[:], op0=OP.add, op1=OP.mult)
                h2 = wp.tile([L, B], F32, tag="h2", name="h2")
                nc.vector.tensor_tensor(h2[:], h1[:], aw[:], op=OP.add)
                p1 = wp.tile([L, B], F32, tag="p1", name="p1")
                nc.vector.tensor_tensor(p1[:], g[:], h2[:], op=OP.mult)
                nc.vector.tensor_tensor(ys[nxt][:], ys[cur][:], p1[:],
                                        op=OP.subtract)

            # ---- final transform ----
            fin = n_tp % 2
            psB = pp.tile([128, 2 * B], F32, tag="psB", name="psB")
            for m in range(2):
                sl = psB[:, m * B:(m + 1) * B]
                ms = slice(m * 128, (m + 1) * 128)
                mm(sl, w["tz1_b"][:, ms], ones_row[:], start=True, stop=False)
                mm(sl, w["tz1_k0"][:, ms], ym[fin][:], start=False, stop=False)
                mm(sl, w["tz1_k1"][:, ms], ys[fin][:], start=False, stop=True)
            h_tz = wp.tile([128, 2 * B], B16, tag="h_ode", name="h_tz")
            nc.scalar.activation(h_tz[:], psB[:], AF.Tanh)
            psE = pp.tile([128, 2 * B], F32, tag="psE", name="psE2")
            for m in range(2):
                sl = psE[:, m * B:(m + 1) * B]
                ms = slice(m * 128, (m + 1) * 128)
                mm(sl, w["tz2_k0"][:, ms], h_tz[:, 0:B], start=True, stop=False)
                mm(sl, w["tz2_k1"][:, ms], h_tz[:, B:], start=False, stop=True)
            o_m = wp.tile([L, B], F32, tag="o_m", name="o_m")
            nc.scalar.activation(o_m[:], psE[:, 0:B], AF.Identity,
                                 bias=w["tz2_bm"][:, 0:1])
            o_s = wp.tile([L, B], F32, tag="o_s", name="o_s")
            nc.scalar.activation(o_s[:], psE[:, B:], AF.Abs,
                                 bias=w["tz2_bs"][:, 0:1])
            nc.sync.dma_start(d_om[:], o_m[:])
            nc.sync.dma_start(d_os[:], o_s[:])

    nc.compile()
    return nc


# --------------------------------------------------------------------------
# host-side packing
# --------------------------------------------------------------------------
def _dt_variants(obs, n_tp):
    F = np.float32
    dd = (obs[:-1] - obs[1:])[::-1]
    dts = np.concatenate([np.full((1,), -0.01, F), dd]).astype(F)
    uniq, vids = np.unique(dts, return_inverse=True)
    return uniq, tuple(int(v) for v in vids)


def _prep_in_maps(inputs, n_tp):
    F = np.float32
    d = {k: np.ascontiguousarray(np.asarray(v, F)) for k, v in inputs.items()}
    obs = d["obs_tps"][:n_tp]
    data = d["data"][:, :n_tp]

    uniq, vids = _dt_variants(obs, n_tp)

    # x slab: [t, c, subj] reversed in time; row 0 = 0.5 * (any-observed)
    xr = data.transpose(1, 2, 0)[::-1]                    # [t, 64, subj]
    m_row = F(0.5) * (xr[:, HALF:].sum(axis=1, keepdims=True) > 0)  # [t,1,subj]
    x_rev = np.concatenate([m_row, xr], axis=1).astype(BF)  # [t, 65, subj]
    x_rev = np.ascontiguousarray(x_rev)

    ns_w1s = d["ns_w1"].copy()
    ns_w1s[:2 * L] *= F(0.5)

    def kx(w1, b1):
        # row 0 = 0 (mask row), rows 1..64 = x weights, row 65 = bias
        return np.vstack([np.zeros((1, w1.shape[1]), F), w1[2 * L:], b1[None, :]])

    bf = {
        "ug1_k0": d["ug_w1"][:L], "ug1_k1": d["ug_w1"][L:2 * L],
        "ug1_kx": kx(d["ug_w1"], d["ug_b1"]),
        "rg1_k0": d["rg_w1"][:L], "rg1_k1": d["rg_w1"][L:2 * L],
        "rg1_kx": kx(d["rg_w1"], d["rg_b1"]),
        "ns1_k0": ns_w1s[:L], "ns1_k1": ns_w1s[L:2 * L],
        "ns1_kx": kx(d["ns_w1"], d["ns_b1"]),
        "ode1_w": d["ode_w1"],
        "ns2_k0": d["ns_w2"][:128], "ns2_k1": d["ns_w2"][128:],
        "ns2_bm16": d["ns_b2"][None, :L],
        "neg_eye": -np.eye(L, dtype=F),
        "tz1_k0": d["tz_w1"][:L], "tz1_k1": d["tz_w1"][L:],
        "tz1_b": d["tz_b1"][None, :],
        "tz2_k0": d["tz_w2"][:128], "tz2_k1": d["tz_w2"][128:],
    }
    for u, dtv in enumerate(uniq):
        bf[f"o2b_{u}"] = d["ode_b2"][None, :] * dtv
    bf["o1br0"] = d["ode_b1"][None, :128]
    bf["o1br1"] = d["ode_b1"][None, 128:]
    shared = {k: np.ascontiguousarray(v.astype(BF)) for k, v in bf.items()}
    # fp8 DoubleRow stationaries: [part, ktile, M] with ktile = (rows 0:128,
    # rows 128:256) of the K=256 layer-2 weights
    F8 = ml_dtypes.float8_e4m3fn

    def k01(w2):
        return np.ascontiguousarray(
            np.stack([w2[:128], w2[128:]], axis=1).astype(F8))

    shared["ug2_k01"] = k01(d["ug_w2"])
    shared["rg2_k01"] = k01(d["rg_w2"])
    for u, dtv in enumerate(uniq):
        shared[f"o2k01_{u}"] = k01(d["ode_w2"] * dtv)
    shared["ug2_bc"] = np.ascontiguousarray(d["ug_b2"][:, None] * F(0.5))
    shared["rg2_bc"] = np.ascontiguousarray(d["rg_b2"][:, None] * F(0.5))
    shared["ns2_bs"] = np.ascontiguousarray(d["ns_b2"][L:, None])
    shared["tz2_bm"] = np.ascontiguousarray(d["tz_b2"][:L, None])
    shared["tz2_bs"] = np.ascontiguousarray(d["tz_b2"][L:, None])

    in_maps = []
    for c in range(N_CORES):
        m = dict(shared)
        m["x_rev"] = np.ascontiguousarray(x_rev[:, :, c * B:(c + 1) * B])
        in_maps.append(m)
    return in_maps


def kernel(**inputs):
    from concourse.bass_utils import run_bass_kernel_spmd

    obs = np.asarray(inputs["obs_tps"], np.float32)[:N_TP]
    _, vids = _dt_variants(obs, N_TP)
    key = (N_TP, SP_ITERS, vids)
    if key not in _CACHE:
        _CACHE[key] = _build(N_TP, SP_ITERS, vids)
    nc = _CACHE[key]

    in_maps = _prep_in_maps(inputs, N_TP)
    res = run_bass_kernel_spmd(nc, in_maps, list(range(N_CORES)))
    outs = res.results

    mean = np.empty((1, N_SUBJ, L), np.float32)
    std = np.empty((1, N_SUBJ, L), np.float32)
    for c in range(N_CORES):
        mean[0, c * B:(c + 1) * B] = outs[c]["out_m"].T
        std[0, c * B:(c + 1) * B] = outs[c]["out_s"].T
    return mean, std


# revision 57
# speedup vs baseline: 1.0234x; 1.0234x over previous
"""Trainium2 Bass kernel for an ODE-RNN encoder (z0 posterior).

Model: 128-step reversed-time GRU-like recurrence with an Euler ODE step on
the mean channel, then a final transform producing (mean_z0, std_z0).

Strategy: data-parallel over the subject (batch) dim across 8 NeuronCores,
weights replicated.  Everything runs on-chip in a transposed layout
([feature, batch], batch=256 on the free dim).  Key points vs a naive port:
- matmul operands and the recurrent state are bf16 (fp32 PSUM accumulate).
- biases ride a ones-row appended to the streamed x tile (layer-1 nets),
  ACT per-partition bias vectors, or K=1 matmuls; zero per-step bias ops.
- the Euler step is folded into the ode2 weights: dt takes very few distinct
  values over the scan, so dt*ode_w2 / dt*ode_b2 are pre-baked per distinct
  value and Yode = Ym + psum(ode2_dt) is a single DVE add (no ACT hop).
- the reset-gate application r.Y = 0.5(1+tanh(zR/2)).Y uses 0.5-pre-scaled
  ns1 weights and am2 = (1+T).Yode as one scalar_tensor_tensor op, so ns1
  costs only K=66 (x) + 2x K=128 (state) matmuls.
- the observation mask m is computed on the host, packed as a 0.5*m row in
  the x stream, and broadcast across partitions with a K=1 matmul; the gate
  factor  -G = (tanh(zU/2) - 1)*(0.5 m)  is one STT op and the blends use
  `subtract` to absorb the sign.
- sigmoid(z) = 0.5 + 0.5*tanh(z/2) keeps every transcendental in the
  resident `exp_and_others` ACT table set (no per-step table switches).
- softplus(x) = log1p(exp(x)) via one Newton step: with w = exp(-|x|) and
  seed y0 = relu(x) + ln2*w, the correction (1+e^x)e^{-y0} collapses
  EXACTLY to (1+w)*2^{-w}, so the tail is 3 ACT ops (Abs, Exp, Exp) and a
  short DVE chain (~1.8e-3 max abs err, under the bf16 noise floor).
- TRN2 allows ONE sync wait per instruction; Bacc legalizes the rest, but
  K=1 dummy matmuls + accumulation-group ordering keep the PE free of
  multi-wait event-semaphore preambles in the steady state.
"""
import sys
import numpy as np
import ml_dtypes

for _p in ("/opt/trn_rl_repo", "/root/.axon_site/_ro/trn_rl_repo"):
    if _p not in sys.path:
        sys.path.append(_p)

N_SUBJ, N_TP, INPUT_DIM, LATENT, N_UNIT = 2048, 128, 64, 128, 256
HALF = INPUT_DIM // 2
N_CORES = 8
B = N_SUBJ // N_CORES          # 256 subjects per core (free dim)
L = LATENT
SP_ITERS = 1                   # kept for test.py compat (cache key)
LN2 = float(np.log(2.0))
BF = ml_dtypes.bfloat16

_CACHE = {}


# --------------------------------------------------------------------------
# Bass program
# --------------------------------------------------------------------------
def _build(n_tp, sp_iters, vids):
    """vids: per-step index into the distinct-dt weight variants."""
    import concourse.mybir as mybir
    from concourse import bacc, tile

    F32 = mybir.dt.float32
    B16 = mybir.dt.bfloat16
    FP8 = mybir.dt.float8e4
    DR = mybir.MatmulPerfMode.DoubleRow
    AF = mybir.ActivationFunctionType
    OP = mybir.AluOpType

    n_var = max(vids) + 1

    # Bacc (not plain Bass): its compile() legalizes the TRN2 one-sync-wait-
    # per-instruction limit (event-semaphore splitting, matmul-wait moves).
    nc = bacc.Bacc(None)

    # ---- DRAM I/O ----
    # x slab per step: row 0 = 0.5*mask-observed, rows 1..64 data
    d_x = nc.dram_tensor("x_rev", [n_tp, INPUT_DIM + 1, B], B16,
                         kind="ExternalInput")

    bspec = {  # bf16 weights (matmul operands)
        "ug1_k0": [L, N_UNIT], "ug1_k1": [L, N_UNIT], "ug1_kx": [INPUT_DIM + 2, N_UNIT],
        "rg1_k0": [L, N_UNIT], "rg1_k1": [L, N_UNIT], "rg1_kx": [INPUT_DIM + 2, N_UNIT],
        "ns1_k0": [L, N_UNIT], "ns1_k1": [L, N_UNIT], "ns1_kx": [INPUT_DIM + 2, N_UNIT],
        "ode1_w": [L, N_UNIT],
        "ns2_k0": [128, 2 * L], "ns2_k1": [128, 2 * L], "ns2_bm16": [1, L],
        "neg_eye": [L, L],
        "tz1_k0": [L, N_UNIT], "tz1_k1": [L, N_UNIT], "tz1_b": [1, N_UNIT],
        "tz2_k0": [128, 2 * L], "tz2_k1": [128, 2 * L],
    }
    # fp8 DoubleRow weights: K=256 reductions in one PE instruction (2 rows
    # per cycle).  Host-validated: fp8 on the gate layer-2 and the dt-baked
    # ode layer-2 keeps end-to-end error ~9e-3 (budget 2e-2); ns1/ns2 in fp8
    # would blow it.
    f8spec = {"ug2_k01": [128, 2, L], "rg2_k01": [128, 2, L]}
    for u in range(n_var):  # dt-baked ode layer-2 weights + dt*b2 rows
        f8spec[f"o2k01_{u}"] = [128, 2, L]
        bspec[f"o2b_{u}"] = [1, L]
    fspec = {  # fp32 per-partition columns (ACT bias vectors)
        "ode1_bc": [128, 2], "ug2_bc": [128, 1], "rg2_bc": [128, 1],
        "ns2_bs": [128, 1], "tz2_bm": [128, 1], "tz2_bs": [128, 1],
    }
    d_w = {k: nc.dram_tensor(k, v, B16, kind="ExternalInput") for k, v in bspec.items()}
    d_w.update({k: nc.dram_tensor(k, v, FP8, kind="ExternalInput")
                for k, v in f8spec.items()})
    d_w.update({k: nc.dram_tensor(k, v, F32, kind="ExternalInput")
                for k, v in fspec.items()})

    d_om = nc.dram_tensor("out_m", [L, B], F32, kind="ExternalOutput")
    d_os = nc.dram_tensor("out_s", [L, B], F32, kind="ExternalOutput")

    with tile.TileContext(nc) as tc:
        with (
            tc.tile_pool(name="const", bufs=1) as cp,
            tc.tile_pool(name="work", bufs=3) as wp,
            tc.tile_pool(name="ps", bufs=1, space="PSUM") as pp,
        ):
            # ---- resident constants / weights ----
            w = {}
            for k, shp in bspec.items():
                w[k] = cp.tile(shp, B16, tag=k, name=k)
                nc.sync.dma_start(w[k][:], d_w[k][:])
            for k, shp in f8spec.items():
                w[k] = cp.tile(shp, FP8, tag=k, name=k)
                nc.sync.dma_start(w[k][:], d_w[k][:])
            for k, shp in fspec.items():
                w[k] = cp.tile(shp, F32, tag=k, name=k)
                nc.sync.dma_start(w[k][:], d_w[k][:])
            ones_row = cp.tile([1, B], B16, tag="ones_row", name="ones_row")
            nc.vector.memset(ones_row[:], 1.0)
            ones1 = cp.tile([1, 128], B16, tag="ones1", name="ones1")
            nc.vector.memset(ones1[:], 1.0)

            xbufs = []
            for j in range(3):
                xb = cp.tile([INPUT_DIM + 2, B], B16, tag=f"xb{j}", name=f"xb{j}")
                # rows 0..64 are DMA-overwritten each step; row 65 stays 1.0
                # (memset must start at a 32-aligned partition, so cover all)
                nc.vector.memset(xb[:], 1.0)
                xbufs.append(xb)

            # state lives in bf16 (matmul-input rounding dominates anyway)
            ym = [cp.tile([L, B], B16, tag=f"ym{i}", name=f"ym{i}") for i in range(2)]
            ys = [cp.tile([L, B], B16, tag=f"ys{i}", name=f"ys{i}") for i in range(2)]
            nc.vector.memset(ym[0][:], 0.0)
            nc.vector.memset(ys[0][:], 0.0)

            mm = nc.tensor.matmul

            # Warm the PE's clock past every weight DMA with K=1 dummy
            # matmuls so steady-state matmuls only wait on one producer.
            scr = pp.tile([1, 16], F32, tag="scr", name="scr")
            for k in bspec:
                mm(scr[0:1, 0:1], w[k][0:1, 0:1], w[k][0:1, 1:2],
                   start=True, stop=True)
            for k in f8spec:
                mm(scr[0:1, 0:1], w[k][0:1, 0:1, 0:1], w[k][0:1, 0:1, 1:2],
                   start=True, stop=True)
            # DVE/ACT read fp32 DMA-produced columns: warm those clocks too
            nf = len(fspec)
            warm_dv = cp.tile([1, 2 * nf], F32, tag="warm_dv", name="warm_dv")
            for j, k in enumerate(fspec):
                nc.vector.tensor_copy(warm_dv[0:1, j:j + 1], w[k][0:1, 0:1])
                nc.scalar.copy(warm_dv[0:1, nf + j:nf + j + 1], w[k][0:1, 0:1])

            # first x slab
            nc.sync.dma_start(xbufs[0][:INPUT_DIM + 1, :], d_x[0])

            # ---- the recurrence ----
            from concourse.tile_rust import add_dep_helper
            cc = float(np.float32(1e-6) - np.float32(1.0))
            prev_vx = None
            for t in range(n_tp):
                cur, nxt = t % 2, (t + 1) % 2
                xb = xbufs[t % 3]
                u = vids[t]
                if t + 1 < n_tp:  # prefetch next step's x slab
                    nc.sync.dma_start(xbufs[(t + 1) % 3][:INPUT_DIM + 1, :],
                                      d_x[t + 1])
                # absorb this step's x-DMA wait into a K=1 dummy
                mm(scr[0:1, 0:1], xb[0:1, 0:1], xb[0:1, 1:2], start=True, stop=True)

                # One start=True per PSUM bank per step (it clears the whole
                # bank's has_written bits); every other matmul accumulates or
                # first-touch-overwrites per element, so groups can interleave
                # freely.  x-only matmuls go first: they are ready before the
                # previous step's state tail finishes, keeping the PE fed.
                psA = pp.tile([128, 4 * B], F32, tag="psA", name="psA")
                psC = pp.tile([128, 2 * B], F32, tag="psC", name="psC")
                psF = pp.tile([128, 2 * B], F32, tag="psF", name="psF")
                # dt*ode_b2 broadcast opens the psF bank (always-ready K=1)
                mm(psF[:, 0:B], w[f"o2b_{u}"][:], ones_row[:],
                   start=True, stop=False)
                # host-computed 0.5*mask row broadcast to all partitions
                mm(psF[:, B:], ones1[:], xb[0:1, :],
                   start=False, stop=False)
                for gi, net in ((1, "rg1"), (0, "ug1")):
                    for m in range(2):
                        sl = psA[:, (2 * gi + m) * B:(2 * gi + m + 1) * B]
                        ms = slice(m * 128, (m + 1) * 128)
                        mm(sl, w[net + "_kx"][:, ms], xb[:],
                           start=(m == 0), stop=False)
                for m in range(2):
                    ms = slice(m * 128, (m + 1) * 128)
                    mm(psC[:, m * B:(m + 1) * B], w["ns1_kx"][:, ms], xb[:],
                       start=(m == 0), stop=False)

                # ODE hidden: tanh(ode_w1^T @ Ym + b1), fp8 out; the whole
                # dt-baked K=256 ode layer-2 is ONE fp8 DoubleRow matmul
                psB = pp.tile([128, 2 * B], F32, tag="psB", name="psB")
                h_ode = wp.tile([128, 2, B], FP8, tag="h_ode", name="h_ode")
                for m in range(2):
                    sl = psB[:, m * B:(m + 1) * B]
                    ms = slice(m * 128, (m + 1) * 128)
                    mm(sl, w["ode1_w"][:, ms], ym[cur][:], start=(m == 0), stop=(m == 1))
                    i_ho = nc.scalar.activation(h_ode[:, m:m + 1, :], sl, AF.Tanh,
                                                bias=w["ode1_bc"][:, m:m + 1])
                    if prev_vx is not None:
                        # keep the ode tanhs out of the previous step's
                        # wx->vx ACT slot (the std-cycle exp chain)
                        add_dep_helper(i_ho.ins, prev_vx.ins, False,
                                       "cycle ACT priority")
                mm(psF[:, 0:B], w[f"o2k01_{u}"][:], h_ode[:],
                   start=False, stop=True, perf_mode=DR)

                # Yode = Ym + dt*(ode_out + b2), dt baked into o2k*/o2b:
                # a single DVE add off the PSUM accumulator
                yode = wp.tile([L, B], B16, tag="yode", name="yode")
                nc.vector.tensor_tensor(yode[:], psF[:, 0:B], ym[cur][:], op=OP.add)

                # gate layer 1 remaining k-tiles; yode (ready first) then ys,
                # per-group contiguous so each m-half completes as soon as its
                # last input lands and its tanh can fire.  The rg m0 pair is
                # pinned to run first: its tanh is on the critical std cycle,
                # and the scheduler otherwise queues rg_k1m0 behind ug work
                # (~1.7us of cycle per step).
                i_g1 = {}
                for gi, net in ((1, "rg1"), (0, "ug1")):
                    for m in range(2):
                        sl = psA[:, (2 * gi + m) * B:(2 * gi + m + 1) * B]
                        ms = slice(m * 128, (m + 1) * 128)
                        i_g1[net, m, 0] = mm(sl, w[net + "_k0"][:, ms], yode[:],
                                             start=False, stop=False)
                        i_g1[net, m, 1] = mm(sl, w[net + "_k1"][:, ms], ys[cur][:],
                                             start=False, stop=(m == 1))


                # layer 2 per gate; rg (reset gate) first: the critical
                # chain runs through R -> as2 -> ns1, U is only needed at
                # the final blend.  rg hidden tanh split per half for the
                # chain; ug hidden batched into one ACT op (off-chain).
                h_g1 = wp.tile([128, 4, B], FP8, tag="h_g1", name="h_g1")
                psD = pp.tile([128, 2 * B], F32, tag="psD", name="psD")
                t_ur = wp.tile([128, 2 * B], B16, tag="t_ur", name="t_ur")
                # one batched tanh: the DoubleRow rg2 matmul consumes both
                # halves at once, so splitting buys no early start and the
                # batch saves ~270ns of serial ACT on the cycle
                nc.scalar.activation(h_g1[:, 2:4, :], psA[:, 2 * B:4 * B],
                                     AF.Tanh)
                mm(psD[:, B:], w["rg2_k01"][:], h_g1[:, 2:4, :],
                   start=True, stop=False, perf_mode=DR)
                i_tur_r = nc.scalar.activation(t_ur[:, B:], psD[:, B:], AF.Tanh,
                                               bias=w["rg2_bc"][:, 0:1], scale=0.5)
                i_ugh = nc.scalar.activation(h_g1[:, 0:2, :], psA[:, 0:2 * B],
                                             AF.Tanh)
                add_dep_helper(i_ugh.ins, i_tur_r.ins, False, "rg ACT priority")
                mm(psD[:, 0:B], w["ug2_k01"][:], h_g1[:, 0:2, :],
                   start=False, stop=True, perf_mode=DR)
                i_tur_u = nc.scalar.activation(t_ur[:, 0:B], psD[:, 0:B], AF.Tanh,
                                               bias=w["ug2_bc"][:, 0:1], scale=0.5)

                # reset-gate products (ns1 k0/k1 pre-scaled 0.5, so
                # r.Y = 0.5(1+T).Y needs only (1+T).Y here); as2 first: the
                # std channel is the critical cycle
                as2 = wp.tile([L, B], B16, tag="as2", name="as2")
                nc.vector.scalar_tensor_tensor(
                    as2[:], t_ur[:, B:], 1.0, ys[cur][:], op0=OP.add, op1=OP.mult)
                am2 = wp.tile([L, B], B16, tag="am2", name="am2")
                nc.vector.scalar_tensor_tensor(
                    am2[:], t_ur[:, B:], 1.0, yode[:], op0=OP.add, op1=OP.mult)
                for m in range(2):
                    sl = psC[:, m * B:(m + 1) * B]
                    ms = slice(m * 128, (m + 1) * 128)
                    mm(sl, w["ns1_k1"][:, ms], as2[:], start=False, stop=False)
                    mm(sl, w["ns1_k0"][:, ms], am2[:], start=False, stop=(m == 1))

                # new-state layer 2: NM | NS pre-acts.  The NM half also
                # accumulates (+bm - Yode); nosync deps keep the bank's
                # start=True matmul first in the PE schedule.
                h_ns = wp.tile([128, 2 * B], B16, tag="h_ns", name="h_ns")
                psE = pp.tile([128, 2 * B], F32, tag="psE", name="psE")
                # bm (always ready) opens the bank and ne (yode-gated) joins
                # it early in the middle, so only the four h_ns-gated matmuls
                # remain between the tanh and the psE group close that
                # releases the tail's readers
                i_bm = mm(psE[:, 0:B], w["ns2_bm16"][:], ones_row[:],
                          start=True, stop=False)
                i_ne = mm(psE[:, 0:B], w["neg_eye"][:], yode[:],
                          start=False, stop=False)
                add_dep_helper(i_ne.ins, i_bm.ins, False, "bank-start order")
                nc.scalar.activation(h_ns[:], psC[:], AF.Tanh)
                i_k0s = mm(psE[:, B:], w["ns2_k0"][:, 128:], h_ns[:, 0:B],
                           start=False, stop=False)
                add_dep_helper(i_k0s.ins, i_ne.ins, False, "bank-start order")
                mm(psE[:, 0:B], w["ns2_k0"][:, 0:128], h_ns[:, 0:B],
                   start=False, stop=False)
                mm(psE[:, B:], w["ns2_k1"][:, 128:], h_ns[:, B:],
                   start=False, stop=False)
                mm(psE[:, 0:B], w["ns2_k1"][:, 0:128], h_ns[:, B:],
                   start=False, stop=True)

                # -G = (tanh(zU/2) - 1) * 0.5m   (one STT; sign absorbed by
                # `subtract` in the blends)
                g = wp.tile([L, B], F32, tag="g", name="g")
                nc.vector.scalar_tensor_tensor(
                    g[:], t_ur[:, 0:B], 1.0, psF[:, B:], op0=OP.subtract,
                    op1=OP.mult)

                # std tail: softplus(x)=log1p(e^x) via one Newton step.
                # w = exp(-|x|); sp = relu(x) + ln2*w - 1 + (1+w)*2^{-w}.
                # xa is emitted FIRST among the psE readers so the one-wait
                # legalizer gives it the direct PE wait (otherwise it chains
                # through a DVE op and starts ~0.5us late).
                xa = wp.tile([L, B], F32, tag="xa", name="xa")
                nc.scalar.activation(xa[:], psE[:, B:], AF.Abs,
                                     bias=w["ns2_bs"][:, 0:1])
                rl = wp.tile([L, B], F32, tag="rl", name="rl")
                nc.vector.tensor_scalar(rl[:], psE[:, B:], w["ns2_bs"][:, 0:1],
                                        0.0, op0=OP.add, op1=OP.max)

                # mean channel: Ym' = Yode - (-G)*(NM + bm - Yode)
                pm = wp.tile([L, B], F32, tag="pm", name="pm")
                nc.vector.tensor_tensor(pm[:], g[:], psE[:, 0:B], op=OP.mult)
                nc.vector.tensor_tensor(ym[nxt][:], yode[:], pm[:], op=OP.subtract)

                wx = wp.tile([L, B], F32, tag="wx", name="wx")
                nc.scalar.activation(wx[:], xa[:], AF.Exp, scale=-1.0)
                vx = wp.tile([L, B], F32, tag="vx", name="vx")
                i_vx = nc.scalar.activation(vx[:], wx[:], AF.Exp, scale=-LN2)
                prev_vx = i_vx
                h0 = wp.tile([L, B], F32, tag="h0", name="h0")
                nc.vector.scalar_tensor_tensor(
                    h0[:], wx[:], LN2, rl[:], op0=OP.mult, op1=OP.add)
                h1 = wp.tile([L, B], F32, tag="h1", name="h1")
                nc.vector.scalar_tensor_tensor(
                    h1[:], h0[:], cc, ys[cur][:], op0=OP.add, op1=OP.subtract)
                aw = wp.tile([L, B], F32, tag="aw", name="aw")
                nc.vector.scalar_tensor_tensor(
                    aw[:], wx[:], 1.0, vx[:], op0=OP.add, op1=OP.mult)
                h2 = wp.tile([L, B], F32, tag="h2", name="h2")
                nc.vector.tensor_tensor(h2[:], h1[:], aw[:], op=OP.add)
                p1 = wp.tile([L, B], F32, tag="p1", name="p1")
                nc.vector.tensor_tensor(p1[:], g[:], h2[:], op=OP.mult)
                nc.vector.tensor_tensor(ys[nxt][:], ys[cur][:], p1[:],
                                        op=OP.subtract)

            # ---- final transform ----
            fin = n_tp % 2
            psB = pp.tile([128, 2 * B], F32, tag="psB", name="psB")
            for m in range(2):
                sl = psB[:, m * B:(m + 1) * B]
                ms = slice(m * 128, (m + 1) * 128)
                mm(sl, w["tz1_b"][:, ms], ones_row[:], start=True, stop=False)
                mm(sl, w["tz1_k0"][:, ms], ym[fin][:], start=False, stop=False)
                mm(sl, w["tz1_k1"][:, ms], ys[fin][:], start=False, stop=True)
            h_tz = wp.tile([128, 2 * B], B16, tag="h_ode", name="h_tz")
            nc.scalar.activation(h_tz[:], psB[:], AF.Tanh)
            psE = pp.tile([128, 2 * B], F32, tag="psE", name="psE2")
            for m in range(2):
                sl = psE[:, m * B:(m + 1) * B]
                ms = slice(m * 128, (m + 1) * 128)
                mm(sl, w["tz2_k0"][:, ms], h_tz[:, 0:B], start=True, stop=False)
                mm(sl, w["tz2_k1"][:, ms], h_tz[:, B:], start=False, stop=True)
            o_m = wp.tile([L, B], F32, tag="o_m", name="o_m")
            nc.scalar.activation(o_m[:], psE[:, 0:B], AF.Identity,
                                 bias=w["tz2_bm"][:, 0:1])
            o_s = wp.tile([L, B], F32, tag="o_s", name="o_s")
            nc.scalar.activation(o_s[:], psE[:, B:], AF.Abs,
                                 bias=w["tz2_bs"][:, 0:1])
            nc.sync.dma_start(d_om[:], o_m[:])
            nc.sync.dma_start(d_os[:], o_s[:])

    nc.compile()
    return nc


# --------------------------------------------------------------------------
# host-side packing
# --------------------------------------------------------------------------
def _dt_variants(obs, n_tp):
    F = np.float32
    dd = (obs[:-1] - obs[1:])[::-1]
    dts = np.concatenate([np.full((1,), -0.01, F), dd]).astype(F)
    uniq, vids = np.unique(dts, return_inverse=True)
    return uniq, tuple(int(v) for v in vids)


def _prep_in_maps(inputs, n_tp):
    F = np.float32
    d = {k: np.ascontiguousarray(np.asarray(v, F)) for k, v in inputs.items()}
    obs = d["obs_tps"][:n_tp]
    data = d["data"][:, :n_tp]

    uniq, vids = _dt_variants(obs, n_tp)

    # x slab: [t, c, subj] reversed in time; row 0 = 0.5 * (any-observed)
    xr = data.transpose(1, 2, 0)[::-1]                    # [t, 64, subj]
    m_row = F(0.5) * (xr[:, HALF:].sum(axis=1, keepdims=True) > 0)  # [t,1,subj]
    x_rev = np.concatenate([m_row, xr], axis=1).astype(BF)  # [t, 65, subj]
    x_rev = np.ascontiguousarray(x_rev)

    ns_w1s = d["ns_w1"].copy()
    ns_w1s[:2 * L] *= F(0.5)

    def kx(w1, b1):
        # row 0 = 0 (mask row), rows 1..64 = x weights, row 65 = bias
        return np.vstack([np.zeros((1, w1.shape[1]), F), w1[2 * L:], b1[None, :]])

    bf = {
        "ug1_k0": d["ug_w1"][:L], "ug1_k1": d["ug_w1"][L:2 * L],
        "ug1_kx": kx(d["ug_w1"], d["ug_b1"]),
        "rg1_k0": d["rg_w1"][:L], "rg1_k1": d["rg_w1"][L:2 * L],
        "rg1_kx": kx(d["rg_w1"], d["rg_b1"]),
        "ns1_k0": ns_w1s[:L], "ns1_k1": ns_w1s[L:2 * L],
        "ns1_kx": kx(d["ns_w1"], d["ns_b1"]),
        "ode1_w": d["ode_w1"],
        "ns2_k0": d["ns_w2"][:128], "ns2_k1": d["ns_w2"][128:],
        "ns2_bm16": d["ns_b2"][None, :L],
        "neg_eye": -np.eye(L, dtype=F),
        "tz1_k0": d["tz_w1"][:L], "tz1_k1": d["tz_w1"][L:],
        "tz1_b": d["tz_b1"][None, :],
        "tz2_k0": d["tz_w2"][:128], "tz2_k1": d["tz_w2"][128:],
    }
    for u, dtv in enumerate(uniq):
        bf[f"o2b_{u}"] = d["ode_b2"][None, :] * dtv
    shared = {k: np.ascontiguousarray(v.astype(BF)) for k, v in bf.items()}
    # fp8 DoubleRow stationaries: [part, ktile, M] with ktile = (rows 0:128,
    # rows 128:256) of the K=256 layer-2 weights
    F8 = ml_dtypes.float8_e4m3fn

    def k01(w2):
        return np.ascontiguousarray(
            np.stack([w2[:128], w2[128:]], axis=1).astype(F8))

    shared["ug2_k01"] = k01(d["ug_w2"])
    shared["rg2_k01"] = k01(d["rg_w2"])
    for u, dtv in enumerate(uniq):
        shared[f"o2k01_{u}"] = k01(d["ode_w2"] * dtv)
    shared["ode1_bc"] = np.ascontiguousarray(d["ode_b1"].reshape(2, 128).T)
    shared["ug2_bc"] = np.ascontiguousarray(d["ug_b2"][:, None] * F(0.5))
    shared["rg2_bc"] = np.ascontiguousarray(d["rg_b2"][:, None] * F(0.5))
    shared["ns2_bs"] = np.ascontiguousarray(d["ns_b2"][L:, None])
    shared["tz2_bm"] = np.ascontiguousarray(d["tz_b2"][:L, None])
    shared["tz2_bs"] = np.ascontiguousarray(d["tz_b2"][L:, None])

    in_maps = []
    for c in range(N_CORES):
        m = dict(shared)
        m["x_rev"] = np.ascontiguousarray(x_rev[:, :, c * B:(c + 1) * B])
        in_maps.append(m)
    return in_maps


def kernel(**inputs):
    from concourse.bass_utils import run_bass_kernel_spmd

    obs = np.asarray(inputs["obs_tps"], np.float32)[:N_TP]
    _, vids = _dt_variants(obs, N_TP)
    key = (N_TP, SP_ITERS, vids)
    if key not in _CACHE:
        _CACHE[key] = _build(N_TP, SP_ITERS, vids)
    nc = _CACHE[key]

    in_maps = _prep_in_maps(inputs, N_TP)
    res = run_bass_kernel_spmd(nc, in_maps, list(range(N_CORES)))
    outs = res.results

    mean = np.empty((1, N_SUBJ, L), np.float32)
    std = np.empty((1, N_SUBJ, L), np.float32)
    for c in range(N_CORES):
        mean[0, c * B:(c + 1) * B] = outs[c]["out_m"].T
        std[0, c * B:(c + 1) * B] = outs[c]["out_s"].T
    return mean, std


# revision 58
# speedup vs baseline: 1.0242x; 1.0008x over previous
"""Trainium2 Bass kernel for an ODE-RNN encoder (z0 posterior).

Model: 128-step reversed-time GRU-like recurrence with an Euler ODE step on
the mean channel, then a final transform producing (mean_z0, std_z0).

Strategy: data-parallel over the subject (batch) dim across 8 NeuronCores,
weights replicated.  Everything runs on-chip in a transposed layout
([feature, batch], batch=256 on the free dim).  Key points vs a naive port:
- matmul operands and the recurrent state are bf16 (fp32 PSUM accumulate).
- biases ride a ones-row appended to the streamed x tile (layer-1 nets),
  ACT per-partition bias vectors, or K=1 matmuls; zero per-step bias ops.
- the Euler step is folded into the ode2 weights: dt takes very few distinct
  values over the scan, so dt*ode_w2 / dt*ode_b2 are pre-baked per distinct
  value and Yode = Ym + psum(ode2_dt) is a single DVE add (no ACT hop).
- the reset-gate application r.Y = 0.5(1+tanh(zR/2)).Y uses 0.5-pre-scaled
  ns1 weights and am2 = (1+T).Yode as one scalar_tensor_tensor op, so ns1
  costs only K=66 (x) + 2x K=128 (state) matmuls.
- the observation mask m is computed on the host, packed as a 0.5*m row in
  the x stream, and broadcast across partitions with a K=1 matmul; the gate
  factor  -G = (tanh(zU/2) - 1)*(0.5 m)  is one STT op and the blends use
  `subtract` to absorb the sign.
- sigmoid(z) = 0.5 + 0.5*tanh(z/2) keeps every transcendental in the
  resident `exp_and_others` ACT table set (no per-step table switches).
- softplus(x) = log1p(exp(x)) via one Newton step: with w = exp(-|x|) and
  seed y0 = relu(x) + ln2*w, the correction (1+e^x)e^{-y0} collapses
  EXACTLY to (1+w)*2^{-w}, so the tail is 3 ACT ops (Abs, Exp, Exp) and a
  short DVE chain (~1.8e-3 max abs err, under the bf16 noise floor).
- TRN2 allows ONE sync wait per instruction; Bacc legalizes the rest, but
  K=1 dummy matmuls + accumulation-group ordering keep the PE free of
  multi-wait event-semaphore preambles in the steady state.
"""
import sys
import numpy as np
import ml_dtypes

for _p in ("/opt/trn_rl_repo", "/root/.axon_site/_ro/trn_rl_repo"):
    if _p not in sys.path:
        sys.path.append(_p)

N_SUBJ, N_TP, INPUT_DIM, LATENT, N_UNIT = 2048, 128, 64, 128, 256
HALF = INPUT_DIM // 2
N_CORES = 8
B = N_SUBJ // N_CORES          # 256 subjects per core (free dim)
L = LATENT
SP_ITERS = 1                   # kept for test.py compat (cache key)
LN2 = float(np.log(2.0))
BF = ml_dtypes.bfloat16

_CACHE = {}


# --------------------------------------------------------------------------
# Bass program
# --------------------------------------------------------------------------
def _build(n_tp, sp_iters, vids):
    """vids: per-step index into the distinct-dt weight variants."""
    import concourse.mybir as mybir
    from concourse import bacc, tile

    F32 = mybir.dt.float32
    B16 = mybir.dt.bfloat16
    FP8 = mybir.dt.float8e4
    DR = mybir.MatmulPerfMode.DoubleRow
    AF = mybir.ActivationFunctionType
    OP = mybir.AluOpType

    n_var = max(vids) + 1

    # Bacc (not plain Bass): its compile() legalizes the TRN2 one-sync-wait-
    # per-instruction limit (event-semaphore splitting, matmul-wait moves).
    nc = bacc.Bacc(None)

    # ---- DRAM I/O ----
    # x slab per step: row 0 = 0.5*mask-observed, rows 1..64 data
    d_x = nc.dram_tensor("x_rev", [n_tp, INPUT_DIM + 1, B], B16,
                         kind="ExternalInput")

    bspec = {  # bf16 weights (matmul operands)
        "ug1_k0": [L, N_UNIT], "ug1_k1": [L, N_UNIT], "ug1_kx": [INPUT_DIM + 2, N_UNIT],
        "rg1_k0": [L, N_UNIT], "rg1_k1": [L, N_UNIT], "rg1_kx": [INPUT_DIM + 2, N_UNIT],
        "ns1_k0": [L, N_UNIT], "ns1_k1": [L, N_UNIT], "ns1_kx": [INPUT_DIM + 2, N_UNIT],
        "ode1_w": [L, N_UNIT],
        "ns2_k0": [128, 2 * L], "ns2_k1": [128, 2 * L], "ns2_bm16": [1, L],
        "neg_eye": [L, L],
        "tz1_k0": [L, N_UNIT], "tz1_k1": [L, N_UNIT], "tz1_b": [1, N_UNIT],
        "tz2_k0": [128, 2 * L], "tz2_k1": [128, 2 * L],
    }
    # fp8 DoubleRow weights: K=256 reductions in one PE instruction (2 rows
    # per cycle).  Host-validated: fp8 on the gate layer-2 and the dt-baked
    # ode layer-2 keeps end-to-end error ~9e-3 (budget 2e-2); ns1/ns2 in fp8
    # would blow it.
    f8spec = {"ug2_k01": [128, 2, L], "rg2_k01": [128, 2, L]}
    for u in range(n_var):  # dt-baked ode layer-2 weights + dt*b2 rows
        f8spec[f"o2k01_{u}"] = [128, 2, L]
        bspec[f"o2b_{u}"] = [1, L]
    fspec = {  # fp32 per-partition columns (ACT bias vectors)
        "ode1_bc": [128, 2], "ug2_bc": [128, 1], "rg2_bc": [128, 1],
        "ns2_bs": [128, 1], "tz2_bm": [128, 1], "tz2_bs": [128, 1],
    }
    d_w = {k: nc.dram_tensor(k, v, B16, kind="ExternalInput") for k, v in bspec.items()}
    d_w.update({k: nc.dram_tensor(k, v, FP8, kind="ExternalInput")
                for k, v in f8spec.items()})
    d_w.update({k: nc.dram_tensor(k, v, F32, kind="ExternalInput")
                for k, v in fspec.items()})

    d_om = nc.dram_tensor("out_m", [L, B], F32, kind="ExternalOutput")
    d_os = nc.dram_tensor("out_s", [L, B], F32, kind="ExternalOutput")

    with tile.TileContext(nc) as tc:
        with (
            tc.tile_pool(name="const", bufs=1) as cp,
            tc.tile_pool(name="work", bufs=3) as wp,
            tc.tile_pool(name="ps", bufs=1, space="PSUM") as pp,
        ):
            # ---- resident constants / weights ----
            w = {}
            for k, shp in bspec.items():
                w[k] = cp.tile(shp, B16, tag=k, name=k)
                nc.sync.dma_start(w[k][:], d_w[k][:])
            for k, shp in f8spec.items():
                w[k] = cp.tile(shp, FP8, tag=k, name=k)
                nc.sync.dma_start(w[k][:], d_w[k][:])
            for k, shp in fspec.items():
                w[k] = cp.tile(shp, F32, tag=k, name=k)
                nc.sync.dma_start(w[k][:], d_w[k][:])
            ones_row = cp.tile([1, B], B16, tag="ones_row", name="ones_row")
            nc.vector.memset(ones_row[:], 1.0)
            ones1 = cp.tile([1, 128], B16, tag="ones1", name="ones1")
            nc.vector.memset(ones1[:], 1.0)

            xbufs = []
            for j in range(3):
                xb = cp.tile([INPUT_DIM + 2, B], B16, tag=f"xb{j}", name=f"xb{j}")
                # rows 0..64 are DMA-overwritten each step; row 65 stays 1.0
                # (memset must start at a 32-aligned partition, so cover all)
                nc.vector.memset(xb[:], 1.0)
                xbufs.append(xb)

            # state lives in bf16 (matmul-input rounding dominates anyway)
            ym = [cp.tile([L, B], B16, tag=f"ym{i}", name=f"ym{i}") for i in range(2)]
            ys = [cp.tile([L, B], B16, tag=f"ys{i}", name=f"ys{i}") for i in range(2)]
            nc.vector.memset(ym[0][:], 0.0)
            nc.vector.memset(ys[0][:], 0.0)

            mm = nc.tensor.matmul

            # Warm the PE's clock past every weight DMA with K=1 dummy
            # matmuls so steady-state matmuls only wait on one producer.
            scr = pp.tile([1, 16], F32, tag="scr", name="scr")
            for k in bspec:
                mm(scr[0:1, 0:1], w[k][0:1, 0:1], w[k][0:1, 1:2],
                   start=True, stop=True)
            for k in f8spec:
                mm(scr[0:1, 0:1], w[k][0:1, 0:1, 0:1], w[k][0:1, 0:1, 1:2],
                   start=True, stop=True)
            # DVE/ACT read fp32 DMA-produced columns: warm those clocks too
            nf = len(fspec)
            warm_dv = cp.tile([1, 2 * nf], F32, tag="warm_dv", name="warm_dv")
            for j, k in enumerate(fspec):
                nc.vector.tensor_copy(warm_dv[0:1, j:j + 1], w[k][0:1, 0:1])
                nc.scalar.copy(warm_dv[0:1, nf + j:nf + j + 1], w[k][0:1, 0:1])

            # first x slab
            nc.sync.dma_start(xbufs[0][:INPUT_DIM + 1, :], d_x[0])

            # ---- the recurrence ----
            from concourse.tile_rust import add_dep_helper
            cc = float(np.float32(1e-6) - np.float32(1.0))
            prev_vx = None
            for t in range(n_tp):
                cur, nxt = t % 2, (t + 1) % 2
                xb = xbufs[t % 3]
                u = vids[t]
                if t + 1 < n_tp:  # prefetch next step's x slab
                    nc.sync.dma_start(xbufs[(t + 1) % 3][:INPUT_DIM + 1, :],
                                      d_x[t + 1])
                # absorb this step's x-DMA wait into a K=1 dummy
                mm(scr[0:1, 0:1], xb[0:1, 0:1], xb[0:1, 1:2], start=True, stop=True)

                # One start=True per PSUM bank per step (it clears the whole
                # bank's has_written bits); every other matmul accumulates or
                # first-touch-overwrites per element, so groups can interleave
                # freely.  x-only matmuls go first: they are ready before the
                # previous step's state tail finishes, keeping the PE fed.
                psA = pp.tile([128, 4 * B], F32, tag="psA", name="psA")
                psC = pp.tile([128, 2 * B], F32, tag="psC", name="psC")
                psF = pp.tile([128, 2 * B], F32, tag="psF", name="psF")
                # dt*ode_b2 broadcast opens the psF bank (always-ready K=1)
                mm(psF[:, 0:B], w[f"o2b_{u}"][:], ones_row[:],
                   start=True, stop=False)
                # host-computed 0.5*mask row broadcast to all partitions
                mm(psF[:, B:], ones1[:], xb[0:1, :],
                   start=False, stop=False)
                for gi, net in ((1, "rg1"), (0, "ug1")):
                    for m in range(2):
                        sl = psA[:, (2 * gi + m) * B:(2 * gi + m + 1) * B]
                        ms = slice(m * 128, (m + 1) * 128)
                        mm(sl, w[net + "_kx"][:, ms], xb[:],
                           start=(m == 0), stop=False)
                for m in range(2):
                    ms = slice(m * 128, (m + 1) * 128)
                    mm(psC[:, m * B:(m + 1) * B], w["ns1_kx"][:, ms], xb[:],
                       start=(m == 0), stop=False)

                # ODE hidden: tanh(ode_w1^T @ Ym + b1), fp8 out; the whole
                # dt-baked K=256 ode layer-2 is ONE fp8 DoubleRow matmul
                psB = pp.tile([128, 2 * B], F32, tag="psB", name="psB")
                h_ode = wp.tile([128, 2, B], FP8, tag="h_ode", name="h_ode")
                for m in range(2):
                    sl = psB[:, m * B:(m + 1) * B]
                    ms = slice(m * 128, (m + 1) * 128)
                    mm(sl, w["ode1_w"][:, ms], ym[cur][:], start=(m == 0), stop=(m == 1))
                    nc.scalar.activation(h_ode[:, m:m + 1, :], sl, AF.Tanh,
                                         bias=w["ode1_bc"][:, m:m + 1])
                mm(psF[:, 0:B], w[f"o2k01_{u}"][:], h_ode[:],
                   start=False, stop=True, perf_mode=DR)

                # Yode = Ym + dt*(ode_out + b2), dt baked into o2k*/o2b:
                # a single DVE add off the PSUM accumulator
                yode = wp.tile([L, B], B16, tag="yode", name="yode")
                nc.vector.tensor_tensor(yode[:], psF[:, 0:B], ym[cur][:], op=OP.add)

                # gate layer 1 remaining k-tiles; yode (ready first) then ys,
                # per-group contiguous so each m-half completes as soon as its
                # last input lands and its tanh can fire.  The rg m0 pair is
                # pinned to run first: its tanh is on the critical std cycle,
                # and the scheduler otherwise queues rg_k1m0 behind ug work
                # (~1.7us of cycle per step).
                i_g1 = {}
                for gi, net in ((1, "rg1"), (0, "ug1")):
                    for m in range(2):
                        sl = psA[:, (2 * gi + m) * B:(2 * gi + m + 1) * B]
                        ms = slice(m * 128, (m + 1) * 128)
                        i_g1[net, m, 0] = mm(sl, w[net + "_k0"][:, ms], yode[:],
                                             start=False, stop=False)
                        i_g1[net, m, 1] = mm(sl, w[net + "_k1"][:, ms], ys[cur][:],
                                             start=False, stop=(m == 1))


                # layer 2 per gate; rg (reset gate) first: the critical
                # chain runs through R -> as2 -> ns1, U is only needed at
                # the final blend.  rg hidden tanh split per half for the
                # chain; ug hidden batched into one ACT op (off-chain).
                h_g1 = wp.tile([128, 4, B], FP8, tag="h_g1", name="h_g1")
                psD = pp.tile([128, 2 * B], F32, tag="psD", name="psD")
                t_ur = wp.tile([128, 2 * B], B16, tag="t_ur", name="t_ur")
                # one batched tanh: the DoubleRow rg2 matmul consumes both
                # halves at once, so splitting buys no early start and the
                # batch saves ~270ns of serial ACT on the cycle
                nc.scalar.activation(h_g1[:, 2:4, :], psA[:, 2 * B:4 * B],
                                     AF.Tanh)
                mm(psD[:, B:], w["rg2_k01"][:], h_g1[:, 2:4, :],
                   start=True, stop=False, perf_mode=DR)
                i_tur_r = nc.scalar.activation(t_ur[:, B:], psD[:, B:], AF.Tanh,
                                               bias=w["rg2_bc"][:, 0:1], scale=0.5)
                i_ugh = nc.scalar.activation(h_g1[:, 0:2, :], psA[:, 0:2 * B],
                                             AF.Tanh)
                add_dep_helper(i_ugh.ins, i_tur_r.ins, False, "rg ACT priority")
                mm(psD[:, 0:B], w["ug2_k01"][:], h_g1[:, 0:2, :],
                   start=False, stop=True, perf_mode=DR)
                i_tur_u = nc.scalar.activation(t_ur[:, 0:B], psD[:, 0:B], AF.Tanh,
                                               bias=w["ug2_bc"][:, 0:1], scale=0.5)

                # reset-gate products (ns1 k0/k1 pre-scaled 0.5, so
                # r.Y = 0.5(1+T).Y needs only (1+T).Y here); as2 first: the
                # std channel is the critical cycle
                as2 = wp.tile([L, B], B16, tag="as2", name="as2")
                nc.vector.scalar_tensor_tensor(
                    as2[:], t_ur[:, B:], 1.0, ys[cur][:], op0=OP.add, op1=OP.mult)
                am2 = wp.tile([L, B], B16, tag="am2", name="am2")
                nc.vector.scalar_tensor_tensor(
                    am2[:], t_ur[:, B:], 1.0, yode[:], op0=OP.add, op1=OP.mult)
                for m in range(2):
                    sl = psC[:, m * B:(m + 1) * B]
                    ms = slice(m * 128, (m + 1) * 128)
                    mm(sl, w["ns1_k1"][:, ms], as2[:], start=False, stop=False)
                    mm(sl, w["ns1_k0"][:, ms], am2[:], start=False, stop=(m == 1))

                # new-state layer 2: NM | NS pre-acts.  The NM half also
                # accumulates (+bm - Yode); nosync deps keep the bank's
                # start=True matmul first in the PE schedule.
                h_ns = wp.tile([128, 2 * B], B16, tag="h_ns", name="h_ns")
                psE = pp.tile([128, 2 * B], F32, tag="psE", name="psE")
                # bm (always ready) opens the bank and ne (yode-gated) joins
                # it early in the middle, so only the four h_ns-gated matmuls
                # remain between the tanh and the psE group close that
                # releases the tail's readers
                i_bm = mm(psE[:, 0:B], w["ns2_bm16"][:], ones_row[:],
                          start=True, stop=False)
                i_ne = mm(psE[:, 0:B], w["neg_eye"][:], yode[:],
                          start=False, stop=False)
                add_dep_helper(i_ne.ins, i_bm.ins, False, "bank-start order")
                nc.scalar.activation(h_ns[:], psC[:], AF.Tanh)
                i_k0s = mm(psE[:, B:], w["ns2_k0"][:, 128:], h_ns[:, 0:B],
                           start=False, stop=False)
                add_dep_helper(i_k0s.ins, i_ne.ins, False, "bank-start order")
                mm(psE[:, 0:B], w["ns2_k0"][:, 0:128], h_ns[:, 0:B],
                   start=False, stop=False)
                mm(psE[:, B:], w["ns2_k1"][:, 128:], h_ns[:, B:],
                   start=False, stop=False)
                mm(psE[:, 0:B], w["ns2_k1"][:, 0:128], h_ns[:, B:],
                   start=False, stop=True)

                # -G = (tanh(zU/2) - 1) * 0.5m   (one STT; sign absorbed by
                # `subtract` in the blends)
                g = wp.tile([L, B], F32, tag="g", name="g")
                nc.vector.scalar_tensor_tensor(
                    g[:], t_ur[:, 0:B], 1.0, psF[:, B:], op0=OP.subtract,
                    op1=OP.mult)

                # std tail: softplus(x)=log1p(e^x) via one Newton step.
                # w = exp(-|x|); sp = relu(x) + ln2*w - 1 + (1+w)*2^{-w}.
                # xa is emitted FIRST among the psE readers so the one-wait
                # legalizer gives it the direct PE wait (otherwise it chains
                # through a DVE op and starts ~0.5us late).
                xa = wp.tile([L, B], F32, tag="xa", name="xa")
                nc.scalar.activation(xa[:], psE[:, B:], AF.Abs,
                                     bias=w["ns2_bs"][:, 0:1])
                rl = wp.tile([L, B], F32, tag="rl", name="rl")
                nc.vector.tensor_scalar(rl[:], psE[:, B:], w["ns2_bs"][:, 0:1],
                                        0.0, op0=OP.add, op1=OP.max)

                # mean channel: Ym' = Yode - (-G)*(NM + bm - Yode)
                pm = wp.tile([L, B], F32, tag="pm", name="pm")
                nc.vector.tensor_tensor(pm[:], g[:], psE[:, 0:B], op=OP.mult)
                nc.vector.tensor_tensor(ym[nxt][:], yode[:], pm[:], op=OP.subtract)

                wx = wp.tile([L, B], F32, tag="wx", name="wx")
                nc.scalar.activation(wx[:], xa[:], AF.Exp, scale=-1.0)
                vx = wp.tile([L, B], F32, tag="vx", name="vx")
                i_vx = nc.scalar.activation(vx[:], wx[:], AF.Exp, scale=-LN2)
                prev_vx = i_vx
                h0 = wp.tile([L, B], F32, tag="h0", name="h0")
                nc.vector.scalar_tensor_tensor(
                    h0[:], wx[:], LN2, rl[:], op0=OP.mult, op1=OP.add)
                h1 = wp.tile([L, B], F32, tag="h1", name="h1")
                nc.vector.scalar_tensor_tensor(
                    h1[:], h0[:], cc, ys[cur][:], op0=OP.add, op1=OP.subtract)
                aw = wp.tile([L, B], F32, tag="aw", name="aw")
                nc.vector.scalar_tensor_tensor(
                    aw[:], wx[:], 1.0, vx[:], op0=OP.add, op1=OP.mult)
                h2 = wp.tile([L, B], F32, tag="h2", name="h2")
                nc.vector.tensor_tensor(h2[:], h1[:], aw[:], op=OP.add)
                p1 = wp.tile([L, B], F32, tag="p1", name="p1")
                nc.vector.tensor_tensor(p1[:], g[:], h2[:], op=OP.mult)
                nc.vector.tensor_tensor(ys[nxt][:], ys[cur][:], p1[:],
                                        op=OP.subtract)

            # ---- final transform ----
            fin = n_tp % 2
            psB = pp.tile([128, 2 * B], F32, tag="psB", name="psB")
            for m in range(2):
                sl = psB[:, m * B:(m + 1) * B]
                ms = slice(m * 128, (m + 1) * 128)
                mm(sl, w["tz1_b"][:, ms], ones_row[:], start=True, stop=False)
                mm(sl, w["tz1_k0"][:, ms], ym[fin][:], start=False, stop=False)
                mm(sl, w["tz1_k1"][:, ms], ys[fin][:], start=False, stop=True)
            h_tz = wp.tile([128, 2 * B], B16, tag="h_ode", name="h_tz")
            nc.scalar.activation(h_tz[:], psB[:], AF.Tanh)
            psE = pp.tile([128, 2 * B], F32, tag="psE", name="psE2")
            for m in range(2):
                sl = psE[:, m * B:(m + 1) * B]
                ms = slice(m * 128, (m + 1) * 128)
                mm(sl, w["tz2_k0"][:, ms], h_tz[:, 0:B], start=True, stop=False)
                mm(sl, w["tz2_k1"][:, ms], h_tz[:, B:], start=False, stop=True)
            o_m = wp.tile([L, B], F32, tag="o_m", name="o_m")
            nc.scalar.activation(o_m[:], psE[:, 0:B], AF.Identity,
                                 bias=w["tz2_bm"][:, 0:1])
            o_s = wp.tile([L, B], F32, tag="o_s", name="o_s")
            nc.scalar.activation(o_s[:], psE[:, B:], AF.Abs,
                                 bias=w["tz2_bs"][:, 0:1])
            nc.sync.dma_start(d_om[:], o_m[:])
            nc.sync.dma_start(d_os[:], o_s[:])

    nc.compile()
    return nc


# --------------------------------------------------------------------------
# host-side packing
# --------------------------------------------------------------------------
def _dt_variants(obs, n_tp):
    F = np.float32
    dd = (obs[:-1] - obs[1:])[::-1]
    dts = np.concatenate([np.full((1,), -0.01, F), dd]).astype(F)
    uniq, vids = np.unique(dts, return_inverse=True)
    return uniq, tuple(int(v) for v in vids)


def _prep_in_maps(inputs, n_tp):
    F = np.float32
    d = {k: np.ascontiguousarray(np.asarray(v, F)) for k, v in inputs.items()}
    obs = d["obs_tps"][:n_tp]
    data = d["data"][:, :n_tp]

    uniq, vids = _dt_variants(obs, n_tp)

    # x slab: [t, c, subj] reversed in time; row 0 = 0.5 * (any-observed)
    xr = data.transpose(1, 2, 0)[::-1]                    # [t, 64, subj]
    m_row = F(0.5) * (xr[:, HALF:].sum(axis=1, keepdims=True) > 0)  # [t,1,subj]
    x_rev = np.concatenate([m_row, xr], axis=1).astype(BF)  # [t, 65, subj]
    x_rev = np.ascontiguousarray(x_rev)

    ns_w1s = d["ns_w1"].copy()
    ns_w1s[:2 * L] *= F(0.5)

    def kx(w1, b1):
        # row 0 = 0 (mask row), rows 1..64 = x weights, row 65 = bias
        return np.vstack([np.zeros((1, w1.shape[1]), F), w1[2 * L:], b1[None, :]])

    bf = {
        "ug1_k0": d["ug_w1"][:L], "ug1_k1": d["ug_w1"][L:2 * L],
        "ug1_kx": kx(d["ug_w1"], d["ug_b1"]),
        "rg1_k0": d["rg_w1"][:L], "rg1_k1": d["rg_w1"][L:2 * L],
        "rg1_kx": kx(d["rg_w1"], d["rg_b1"]),
        "ns1_k0": ns_w1s[:L], "ns1_k1": ns_w1s[L:2 * L],
        "ns1_kx": kx(d["ns_w1"], d["ns_b1"]),
        "ode1_w": d["ode_w1"],
        "ns2_k0": d["ns_w2"][:128], "ns2_k1": d["ns_w2"][128:],
        "ns2_bm16": d["ns_b2"][None, :L],
        "neg_eye": -np.eye(L, dtype=F),
        "tz1_k0": d["tz_w1"][:L], "tz1_k1": d["tz_w1"][L:],
        "tz1_b": d["tz_b1"][None, :],
        "tz2_k0": d["tz_w2"][:128], "tz2_k1": d["tz_w2"][128:],
    }
    for u, dtv in enumerate(uniq):
        bf[f"o2b_{u}"] = d["ode_b2"][None, :] * dtv
    shared = {k: np.ascontiguousarray(v.astype(BF)) for k, v in bf.items()}
    # fp8 DoubleRow stationaries: [part, ktile, M] with ktile = (rows 0:128,
    # rows 128:256) of the K=256 layer-2 weights
    F8 = ml_dtypes.float8_e4m3fn

    def k01(w2):
        return np.ascontiguousarray(
            np.stack([w2[:128], w2[128:]], axis=1).astype(F8))

    shared["ug2_k01"] = k01(d["ug_w2"])
    shared["rg2_k01"] = k01(d["rg_w2"])
    for u, dtv in enumerate(uniq):
        shared[f"o2k01_{u}"] = k01(d["ode_w2"] * dtv)
    shared["ode1_bc"] = np.ascontiguousarray(d["ode_b1"].reshape(2, 128).T)
    shared["ug2_bc"] = np.ascontiguousarray(d["ug_b2"][:, None] * F(0.5))
    shared["rg2_bc"] = np.ascontiguousarray(d["rg_b2"][:, None] * F(0.5))
    shared["ns2_bs"] = np.ascontiguousarray(d["ns_b2"][L:, None])
    shared["tz2_bm"] = np.ascontiguousarray(d["tz_b2"][:L, None])
    shared["tz2_bs"] = np.ascontiguousarray(d["tz_b2"][L:, None])

    in_maps = []
    for c in range(N_CORES):
        m = dict(shared)
        m["x_rev"] = np.ascontiguousarray(x_rev[:, :, c * B:(c + 1) * B])
        in_maps.append(m)
    return in_maps


def kernel(**inputs):
    from concourse.bass_utils import run_bass_kernel_spmd

    obs = np.asarray(inputs["obs_tps"], np.float32)[:N_TP]
    _, vids = _dt_variants(obs, N_TP)
    key = (N_TP, SP_ITERS, vids)
    if key not in _CACHE:
        _CACHE[key] = _build(N_TP, SP_ITERS, vids)
    nc = _CACHE[key]

    in_maps = _prep_in_maps(inputs, N_TP)
    res = run_bass_kernel_spmd(nc, in_maps, list(range(N_CORES)))
    outs = res.results

    mean = np.empty((1, N_SUBJ, L), np.float32)
    std = np.empty((1, N_SUBJ, L), np.float32)
    for c in range(N_CORES):
        mean[0, c * B:(c + 1) * B] = outs[c]["out_m"].T
        std[0, c * B:(c + 1) * B] = outs[c]["out_s"].T
    return mean, std


# revision 60
# speedup vs baseline: 1.0388x; 1.0142x over previous
"""Trainium2 Bass kernel for an ODE-RNN encoder (z0 posterior).

Model: 128-step reversed-time GRU-like recurrence with an Euler ODE step on
the mean channel, then a final transform producing (mean_z0, std_z0).

Strategy: data-parallel over the subject (batch) dim across 8 NeuronCores,
weights replicated.  Everything runs on-chip in a transposed layout
([feature, batch], batch=256 on the free dim).  Key points vs a naive port:
- matmul operands and the recurrent state are bf16 (fp32 PSUM accumulate).
- biases ride a ones-row appended to the streamed x tile (layer-1 nets),
  ACT per-partition bias vectors, or K=1 matmuls; zero per-step bias ops.
- the Euler step is folded into the ode2 weights: dt takes very few distinct
  values over the scan, so dt*ode_w2 / dt*ode_b2 are pre-baked per distinct
  value and Yode = Ym + psum(ode2_dt) is a single DVE add (no ACT hop).
- the reset-gate application r.Y = 0.5(1+tanh(zR/2)).Y uses 0.5-pre-scaled
  ns1 weights and am2 = (1+T).Yode as one scalar_tensor_tensor op, so ns1
  costs only K=66 (x) + 2x K=128 (state) matmuls.
- the observation mask m is computed on the host, packed as a 0.5*m row in
  the x stream, and broadcast across partitions with a K=1 matmul; the gate
  factor  -G = (tanh(zU/2) - 1)*(0.5 m)  is one STT op and the blends use
  `subtract` to absorb the sign.
- sigmoid(z) = 0.5 + 0.5*tanh(z/2) keeps every transcendental in the
  resident `exp_and_others` ACT table set (no per-step table switches).
- softplus(x) = log1p(exp(x)) via one Newton step: with w = exp(-|x|) and
  seed y0 = relu(x) + ln2*w, the correction (1+e^x)e^{-y0} collapses
  EXACTLY to (1+w)*2^{-w}, so the tail is 3 ACT ops (Abs, Exp, Exp) and a
  short DVE chain (~1.8e-3 max abs err, under the bf16 noise floor).
- TRN2 allows ONE sync wait per instruction; Bacc legalizes the rest, but
  K=1 dummy matmuls + accumulation-group ordering keep the PE free of
  multi-wait event-semaphore preambles in the steady state.
"""
import sys
import numpy as np
import ml_dtypes

for _p in ("/opt/trn_rl_repo", "/root/.axon_site/_ro/trn_rl_repo"):
    if _p not in sys.path:
        sys.path.append(_p)

N_SUBJ, N_TP, INPUT_DIM, LATENT, N_UNIT = 2048, 128, 64, 128, 256
HALF = INPUT_DIM // 2
N_CORES = 8
B = N_SUBJ // N_CORES          # 256 subjects per core (free dim)
L = LATENT
SP_ITERS = 1                   # kept for test.py compat (cache key)
LN2 = float(np.log(2.0))
BF = ml_dtypes.bfloat16

_CACHE = {}


# --------------------------------------------------------------------------
# Bass program
# --------------------------------------------------------------------------
def _build(n_tp, sp_iters, vids):
    """vids: per-step index into the distinct-dt weight variants."""
    import concourse.mybir as mybir
    from concourse import bacc, tile

    F32 = mybir.dt.float32
    B16 = mybir.dt.bfloat16
    FP8 = mybir.dt.float8e4
    DR = mybir.MatmulPerfMode.DoubleRow
    AF = mybir.ActivationFunctionType
    OP = mybir.AluOpType

    n_var = max(vids) + 1

    # Bacc (not plain Bass): its compile() legalizes the TRN2 one-sync-wait-
    # per-instruction limit (event-semaphore splitting, matmul-wait moves).
    nc = bacc.Bacc(None)

    # ---- DRAM I/O ----
    # x slab per step: row 0 = 0.5*mask-observed, rows 1..64 data
    d_x = nc.dram_tensor("x_rev", [n_tp, INPUT_DIM + 1, B], B16,
                         kind="ExternalInput")

    bspec = {  # bf16 weights (matmul operands)
        "ug1_k0": [L, N_UNIT], "ug1_k1": [L, N_UNIT], "ug1_kx": [INPUT_DIM + 2, N_UNIT],
        "rg1_k0": [L, N_UNIT], "rg1_k1": [L, N_UNIT], "rg1_kx": [INPUT_DIM + 2, N_UNIT],
        "ns1_k0": [L, N_UNIT], "ns1_k1": [L, N_UNIT], "ns1_kx": [INPUT_DIM + 2, N_UNIT],
        "ode1_w": [L, N_UNIT],
        "ns2_k0": [128, 2 * L], "ns2_k1": [128, 2 * L], "ns2_bm16": [1, L],
        "neg_eye": [L, L],
        "tz1_k0": [L, N_UNIT], "tz1_k1": [L, N_UNIT], "tz1_b": [1, N_UNIT],
        "tz2_k0": [128, 2 * L], "tz2_k1": [128, 2 * L],
    }
    # fp8 DoubleRow weights: K=256 reductions in one PE instruction (2 rows
    # per cycle).  Host-validated: fp8 on the gate layer-2 and the dt-baked
    # ode layer-2 keeps end-to-end error ~9e-3 (budget 2e-2); ns1/ns2 in fp8
    # would blow it.
    f8spec = {"ug2_k01": [128, 2, L], "rg2_k01": [128, 2, L]}
    for u in range(n_var):  # dt-baked ode layer-2 weights + dt*b2 rows
        f8spec[f"o2k01_{u}"] = [128, 2, L]
        bspec[f"o2b_{u}"] = [1, L]
    fspec = {  # fp32 per-partition columns (ACT bias vectors)
        "ode1_bc": [128, 2], "ug2_bc": [128, 1], "rg2_bc": [128, 1],
        "ns2_bs": [128, 1], "tz2_bm": [128, 1], "tz2_bs": [128, 1],
    }
    d_w = {k: nc.dram_tensor(k, v, B16, kind="ExternalInput") for k, v in bspec.items()}
    d_w.update({k: nc.dram_tensor(k, v, FP8, kind="ExternalInput")
                for k, v in f8spec.items()})
    d_w.update({k: nc.dram_tensor(k, v, F32, kind="ExternalInput")
                for k, v in fspec.items()})

    d_om = nc.dram_tensor("out_m", [L, B], F32, kind="ExternalOutput")
    d_os = nc.dram_tensor("out_s", [L, B], F32, kind="ExternalOutput")

    with tile.TileContext(nc) as tc:
        with (
            tc.tile_pool(name="const", bufs=1) as cp,
            tc.tile_pool(name="work", bufs=3) as wp,
            tc.tile_pool(name="ps", bufs=1, space="PSUM") as pp,
        ):
            # ---- resident constants / weights ----
            w = {}
            for k, shp in bspec.items():
                w[k] = cp.tile(shp, B16, tag=k, name=k)
                nc.sync.dma_start(w[k][:], d_w[k][:])
            for k, shp in f8spec.items():
                w[k] = cp.tile(shp, FP8, tag=k, name=k)
                nc.sync.dma_start(w[k][:], d_w[k][:])
            for k, shp in fspec.items():
                w[k] = cp.tile(shp, F32, tag=k, name=k)
                nc.sync.dma_start(w[k][:], d_w[k][:])
            ones_row = cp.tile([1, B], B16, tag="ones_row", name="ones_row")
            nc.vector.memset(ones_row[:], 1.0)
            ones1 = cp.tile([1, 128], B16, tag="ones1", name="ones1")
            nc.vector.memset(ones1[:], 1.0)

            xbufs = []
            for j in range(3):
                xb = cp.tile([INPUT_DIM + 2, B], B16, tag=f"xb{j}", name=f"xb{j}")
                # rows 0..64 are DMA-overwritten each step; row 65 stays 1.0
                # (memset must start at a 32-aligned partition, so cover all)
                nc.vector.memset(xb[:], 1.0)
                xbufs.append(xb)

            # state lives in bf16 (matmul-input rounding dominates anyway)
            ym = [cp.tile([L, B], B16, tag=f"ym{i}", name=f"ym{i}") for i in range(2)]
            ys = [cp.tile([L, B], B16, tag=f"ys{i}", name=f"ys{i}") for i in range(2)]
            nc.vector.memset(ym[0][:], 0.0)
            nc.vector.memset(ys[0][:], 0.0)

            mm = nc.tensor.matmul

            # Warm the PE's clock past every weight DMA with K=1 dummy
            # matmuls so steady-state matmuls only wait on one producer.
            scr = pp.tile([1, 16], F32, tag="scr", name="scr")
            for k in bspec:
                mm(scr[0:1, 0:1], w[k][0:1, 0:1], w[k][0:1, 1:2],
                   start=True, stop=True)
            for k in f8spec:
                mm(scr[0:1, 0:1], w[k][0:1, 0:1, 0:1], w[k][0:1, 0:1, 1:2],
                   start=True, stop=True)
            # DVE/ACT read fp32 DMA-produced columns: warm those clocks too
            nf = len(fspec)
            warm_dv = cp.tile([1, 2 * nf], F32, tag="warm_dv", name="warm_dv")
            for j, k in enumerate(fspec):
                nc.vector.tensor_copy(warm_dv[0:1, j:j + 1], w[k][0:1, 0:1])
                nc.scalar.copy(warm_dv[0:1, nf + j:nf + j + 1], w[k][0:1, 0:1])

            # first x slab
            nc.sync.dma_start(xbufs[0][:INPUT_DIM + 1, :], d_x[0])

            # ---- the recurrence ----
            from concourse.tile_rust import add_dep_helper
            cc = float(np.float32(1e-6) - np.float32(1.0))
            for t in range(n_tp):
                cur, nxt = t % 2, (t + 1) % 2
                xb = xbufs[t % 3]
                u = vids[t]
                if t + 1 < n_tp:  # prefetch next step's x slab
                    nc.sync.dma_start(xbufs[(t + 1) % 3][:INPUT_DIM + 1, :],
                                      d_x[t + 1])
                # x is prefetched a full step ahead, so the first kx matmul's
                # DMA wait is already satisfied in steady state: no dummy
                # absorber needed (saves one PE instruction per step)

                # One start=True per PSUM bank per step (it clears the whole
                # bank's has_written bits); every other matmul accumulates or
                # first-touch-overwrites per element, so groups can interleave
                # freely.  x-only matmuls go first: they are ready before the
                # previous step's state tail finishes, keeping the PE fed.
                psA = pp.tile([128, 4 * B], F32, tag="psA", name="psA")
                psC = pp.tile([128, 2 * B], F32, tag="psC", name="psC")
                psF = pp.tile([128, 2 * B], F32, tag="psF", name="psF")
                # dt*ode_b2 broadcast opens the psF bank (always-ready K=1)
                mm(psF[:, 0:B], w[f"o2b_{u}"][:], ones_row[:],
                   start=True, stop=False)
                # host-computed 0.5*mask row broadcast to all partitions
                mm(psF[:, B:], ones1[:], xb[0:1, :],
                   start=False, stop=False)
                for gi, net in ((1, "rg1"), (0, "ug1")):
                    for m in range(2):
                        sl = psA[:, (2 * gi + m) * B:(2 * gi + m + 1) * B]
                        ms = slice(m * 128, (m + 1) * 128)
                        mm(sl, w[net + "_kx"][:, ms], xb[:],
                           start=(m == 0), stop=False)
                for m in range(2):
                    ms = slice(m * 128, (m + 1) * 128)
                    mm(psC[:, m * B:(m + 1) * B], w["ns1_kx"][:, ms], xb[:],
                       start=(m == 0), stop=False)

                # ODE hidden: tanh(ode_w1^T @ Ym + b1), fp8 out; the whole
                # dt-baked K=256 ode layer-2 is ONE fp8 DoubleRow matmul
                psB = pp.tile([128, 2 * B], F32, tag="psB", name="psB")
                h_ode = wp.tile([128, 2, B], FP8, tag="h_ode", name="h_ode")
                for m in range(2):
                    sl = psB[:, m * B:(m + 1) * B]
                    ms = slice(m * 128, (m + 1) * 128)
                    mm(sl, w["ode1_w"][:, ms], ym[cur][:], start=(m == 0), stop=(m == 1))
                    nc.scalar.activation(h_ode[:, m:m + 1, :], sl, AF.Tanh,
                                         bias=w["ode1_bc"][:, m:m + 1])
                mm(psF[:, 0:B], w[f"o2k01_{u}"][:], h_ode[:],
                   start=False, stop=True, perf_mode=DR)

                # Yode = Ym + dt*(ode_out + b2), dt baked into o2k*/o2b:
                # a single DVE add off the PSUM accumulator
                yode = wp.tile([L, B], B16, tag="yode", name="yode")
                nc.vector.tensor_tensor(yode[:], psF[:, 0:B], ym[cur][:], op=OP.add)

                # gate layer 1 remaining k-tiles; yode (ready first) then ys,
                # per-group contiguous so each m-half completes as soon as its
                # last input lands and its tanh can fire.  The rg m0 pair is
                # pinned to run first: its tanh is on the critical std cycle,
                # and the scheduler otherwise queues rg_k1m0 behind ug work
                # (~1.7us of cycle per step).
                i_g1 = {}
                for gi, net in ((1, "rg1"), (0, "ug1")):
                    for m in range(2):
                        sl = psA[:, (2 * gi + m) * B:(2 * gi + m + 1) * B]
                        ms = slice(m * 128, (m + 1) * 128)
                        i_g1[net, m, 0] = mm(sl, w[net + "_k0"][:, ms], yode[:],
                                             start=False, stop=False)
                        i_g1[net, m, 1] = mm(sl, w[net + "_k1"][:, ms], ys[cur][:],
                                             start=False, stop=(m == 1))


                # layer 2 per gate; rg (reset gate) first: the critical
                # chain runs through R -> as2 -> ns1, U is only needed at
                # the final blend.  rg hidden tanh split per half for the
                # chain; ug hidden batched into one ACT op (off-chain).
                h_g1 = wp.tile([128, 4, B], FP8, tag="h_g1", name="h_g1")
                psD = pp.tile([128, 2 * B], F32, tag="psD", name="psD")
                t_ur = wp.tile([128, 2 * B], B16, tag="t_ur", name="t_ur")
                # one batched tanh: the DoubleRow rg2 matmul consumes both
                # halves at once, so splitting buys no early start and the
                # batch saves ~270ns of serial ACT on the cycle
                nc.scalar.activation(h_g1[:, 2:4, :], psA[:, 2 * B:4 * B],
                                     AF.Tanh)
                mm(psD[:, B:], w["rg2_k01"][:], h_g1[:, 2:4, :],
                   start=True, stop=False, perf_mode=DR)
                i_tur_r = nc.scalar.activation(t_ur[:, B:], psD[:, B:], AF.Tanh,
                                               bias=w["rg2_bc"][:, 0:1], scale=0.5)
                i_ugh = nc.scalar.activation(h_g1[:, 0:2, :], psA[:, 0:2 * B],
                                             AF.Tanh)
                add_dep_helper(i_ugh.ins, i_tur_r.ins, False, "rg ACT priority")
                mm(psD[:, 0:B], w["ug2_k01"][:], h_g1[:, 0:2, :],
                   start=False, stop=True, perf_mode=DR)
                i_tur_u = nc.scalar.activation(t_ur[:, 0:B], psD[:, 0:B], AF.Tanh,
                                               bias=w["ug2_bc"][:, 0:1], scale=0.5)

                # reset-gate products (ns1 k0/k1 pre-scaled 0.5, so
                # r.Y = 0.5(1+T).Y needs only (1+T).Y here); as2 first: the
                # std channel is the critical cycle
                as2 = wp.tile([L, B], B16, tag="as2", name="as2")
                nc.vector.scalar_tensor_tensor(
                    as2[:], t_ur[:, B:], 1.0, ys[cur][:], op0=OP.add, op1=OP.mult)
                am2 = wp.tile([L, B], B16, tag="am2", name="am2")
                nc.vector.scalar_tensor_tensor(
                    am2[:], t_ur[:, B:], 1.0, yode[:], op0=OP.add, op1=OP.mult)
                for m in range(2):
                    sl = psC[:, m * B:(m + 1) * B]
                    ms = slice(m * 128, (m + 1) * 128)
                    mm(sl, w["ns1_k1"][:, ms], as2[:], start=False, stop=False)
                    mm(sl, w["ns1_k0"][:, ms], am2[:], start=False, stop=(m == 1))

                # new-state layer 2: NM | NS pre-acts.  The NM half also
                # accumulates (+bm - Yode); nosync deps keep the bank's
                # start=True matmul first in the PE schedule.
                h_ns = wp.tile([128, 2 * B], B16, tag="h_ns", name="h_ns")
                psE = pp.tile([128, 2 * B], F32, tag="psE", name="psE")
                # bm (always ready) opens the bank and ne (yode-gated) joins
                # it early in the middle, so only the four h_ns-gated matmuls
                # remain between the tanh and the psE group close that
                # releases the tail's readers
                i_bm = mm(psE[:, 0:B], w["ns2_bm16"][:], ones_row[:],
                          start=True, stop=False)
                i_ne = mm(psE[:, 0:B], w["neg_eye"][:], yode[:],
                          start=False, stop=False)
                add_dep_helper(i_ne.ins, i_bm.ins, False, "bank-start order")
                nc.scalar.activation(h_ns[:], psC[:], AF.Tanh)
                i_k0s = mm(psE[:, B:], w["ns2_k0"][:, 128:], h_ns[:, 0:B],
                           start=False, stop=False)
                add_dep_helper(i_k0s.ins, i_ne.ins, False, "bank-start order")
                mm(psE[:, 0:B], w["ns2_k0"][:, 0:128], h_ns[:, 0:B],
                   start=False, stop=False)
                mm(psE[:, B:], w["ns2_k1"][:, 128:], h_ns[:, B:],
                   start=False, stop=False)
                mm(psE[:, 0:B], w["ns2_k1"][:, 0:128], h_ns[:, B:],
                   start=False, stop=True)

                # -G = (tanh(zU/2) - 1) * 0.5m   (one STT; sign absorbed by
                # `subtract` in the blends)
                g = wp.tile([L, B], F32, tag="g", name="g")
                nc.vector.scalar_tensor_tensor(
                    g[:], t_ur[:, 0:B], 1.0, psF[:, B:], op0=OP.subtract,
                    op1=OP.mult)

                # std tail: softplus(x)=log1p(e^x) via one Newton step.
                # w = exp(-|x|); sp = relu(x) + ln2*w - 1 + (1+w)*2^{-w}
                rl = wp.tile([L, B], F32, tag="rl", name="rl")
                nc.vector.tensor_scalar(rl[:], psE[:, B:], w["ns2_bs"][:, 0:1],
                                        0.0, op0=OP.add, op1=OP.max)

                # mean channel: Ym' = Yode - (-G)*(NM + bm - Yode)
                pm = wp.tile([L, B], F32, tag="pm", name="pm")
                nc.vector.tensor_tensor(pm[:], g[:], psE[:, 0:B], op=OP.mult)
                nc.vector.tensor_tensor(ym[nxt][:], yode[:], pm[:], op=OP.subtract)

                xa = wp.tile([L, B], F32, tag="xa", name="xa")
                nc.scalar.activation(xa[:], psE[:, B:], AF.Abs,
                                     bias=w["ns2_bs"][:, 0:1])
                wx = wp.tile([L, B], F32, tag="wx", name="wx")
                nc.scalar.activation(wx[:], xa[:], AF.Exp, scale=-1.0)
                vx = wp.tile([L, B], F32, tag="vx", name="vx")
                nc.scalar.activation(vx[:], wx[:], AF.Exp, scale=-LN2)
                h0 = wp.tile([L, B], F32, tag="h0", name="h0")
                nc.vector.scalar_tensor_tensor(
                    h0[:], wx[:], LN2, rl[:], op0=OP.mult, op1=OP.add)
                h1 = wp.tile([L, B], F32, tag="h1", name="h1")
                nc.vector.scalar_tensor_tensor(
                    h1[:], h0[:], cc, ys[cur][:], op0=OP.add, op1=OP.subtract)
                aw = wp.tile([L, B], F32, tag="aw", name="aw")
                nc.vector.scalar_tensor_tensor(
                    aw[:], wx[:], 1.0, vx[:], op0=OP.add, op1=OP.mult)
                h2 = wp.tile([L, B], F32, tag="h2", name="h2")
                nc.vector.tensor_tensor(h2[:], h1[:], aw[:], op=OP.add)
                p1 = wp.tile([L, B], F32, tag="p1", name="p1")
                nc.vector.tensor_tensor(p1[:], g[:], h2[:], op=OP.mult)
                nc.vector.tensor_tensor(ys[nxt][:], ys[cur][:], p1[:],
                                        op=OP.subtract)

            # ---- final transform ----
            fin = n_tp % 2
            psB = pp.tile([128, 2 * B], F32, tag="psB", name="psB")
            for m in range(2):
                sl = psB[:, m * B:(m + 1) * B]
                ms = slice(m * 128, (m + 1) * 128)
                mm(sl, w["tz1_b"][:, ms], ones_row[:], start=True, stop=False)
                mm(sl, w["tz1_k0"][:, ms], ym[fin][:], start=False, stop=False)
                mm(sl, w["tz1_k1"][:, ms], ys[fin][:], start=False, stop=True)
            h_tz = wp.tile([128, 2 * B], B16, tag="h_ode", name="h_tz")
            nc.scalar.activation(h_tz[:], psB[:], AF.Tanh)
            psE = pp.tile([128, 2 * B], F32, tag="psE", name="psE2")
            for m in range(2):
                sl = psE[:, m * B:(m + 1) * B]
                ms = slice(m * 128, (m + 1) * 128)
                mm(sl, w["tz2_k0"][:, ms], h_tz[:, 0:B], start=True, stop=False)
                mm(sl, w["tz2_k1"][:, ms], h_tz[:, B:], start=False, stop=True)
            o_m = wp.tile([L, B], F32, tag="o_m", name="o_m")
            nc.scalar.activation(o_m[:], psE[:, 0:B], AF.Identity,
                                 bias=w["tz2_bm"][:, 0:1])
            o_s = wp.tile([L, B], F32, tag="o_s", name="o_s")
            nc.scalar.activation(o_s[:], psE[:, B:], AF.Abs,
                                 bias=w["tz2_bs"][:, 0:1])
            nc.sync.dma_start(d_om[:], o_m[:])
            nc.sync.dma_start(d_os[:], o_s[:])

    nc.compile()
    return nc


# --------------------------------------------------------------------------
# host-side packing
# --------------------------------------------------------------------------
def _dt_variants(obs, n_tp):
    F = np.float32
    dd = (obs[:-1] - obs[1:])[::-1]
    dts = np.concatenate([np.full((1,), -0.01, F), dd]).astype(F)
    uniq, vids = np.unique(dts, return_inverse=True)
    return uniq, tuple(int(v) for v in vids)


def _prep_in_maps(inputs, n_tp):
    F = np.float32
    d = {k: np.ascontiguousarray(np.asarray(v, F)) for k, v in inputs.items()}
    obs = d["obs_tps"][:n_tp]
    data = d["data"][:, :n_tp]

    uniq, vids = _dt_variants(obs, n_tp)

    # x slab: [t, c, subj] reversed in time; row 0 = 0.5 * (any-observed)
    xr = data.transpose(1, 2, 0)[::-1]                    # [t, 64, subj]
    m_row = F(0.5) * (xr[:, HALF:].sum(axis=1, keepdims=True) > 0)  # [t,1,subj]
    x_rev = np.concatenate([m_row, xr], axis=1).astype(BF)  # [t, 65, subj]
    x_rev = np.ascontiguousarray(x_rev)

    ns_w1s = d["ns_w1"].copy()
    ns_w1s[:2 * L] *= F(0.5)

    def kx(w1, b1):
        # row 0 = 0 (mask row), rows 1..64 = x weights, row 65 = bias
        return np.vstack([np.zeros((1, w1.shape[1]), F), w1[2 * L:], b1[None, :]])

    bf = {
        "ug1_k0": d["ug_w1"][:L], "ug1_k1": d["ug_w1"][L:2 * L],
        "ug1_kx": kx(d["ug_w1"], d["ug_b1"]),
        "rg1_k0": d["rg_w1"][:L], "rg1_k1": d["rg_w1"][L:2 * L],
        "rg1_kx": kx(d["rg_w1"], d["rg_b1"]),
        "ns1_k0": ns_w1s[:L], "ns1_k1": ns_w1s[L:2 * L],
        "ns1_kx": kx(d["ns_w1"], d["ns_b1"]),
        "ode1_w": d["ode_w1"],
        "ns2_k0": d["ns_w2"][:128], "ns2_k1": d["ns_w2"][128:],
        "ns2_bm16": d["ns_b2"][None, :L],
        "neg_eye": -np.eye(L, dtype=F),
        "tz1_k0": d["tz_w1"][:L], "tz1_k1": d["tz_w1"][L:],
        "tz1_b": d["tz_b1"][None, :],
        "tz2_k0": d["tz_w2"][:128], "tz2_k1": d["tz_w2"][128:],
    }
    for u, dtv in enumerate(uniq):
        bf[f"o2b_{u}"] = d["ode_b2"][None, :] * dtv
    shared = {k: np.ascontiguousarray(v.astype(BF)) for k, v in bf.items()}
    # fp8 DoubleRow stationaries: [part, ktile, M] with ktile = (rows 0:128,
    # rows 128:256) of the K=256 layer-2 weights
    F8 = ml_dtypes.float8_e4m3fn

    def k01(w2):
        return np.ascontiguousarray(
            np.stack([w2[:128], w2[128:]], axis=1).astype(F8))

    shared["ug2_k01"] = k01(d["ug_w2"])
    shared["rg2_k01"] = k01(d["rg_w2"])
    for u, dtv in enumerate(uniq):
        shared[f"o2k01_{u}"] = k01(d["ode_w2"] * dtv)
    shared["ode1_bc"] = np.ascontiguousarray(d["ode_b1"].reshape(2, 128).T)
    shared["ug2_bc"] = np.ascontiguousarray(d["ug_b2"][:, None] * F(0.5))
    shared["rg2_bc"] = np.ascontiguousarray(d["rg_b2"][:, None] * F(0.5))
    shared["ns2_bs"] = np.ascontiguousarray(d["ns_b2"][L:, None])
    shared["tz2_bm"] = np.ascontiguousarray(d["tz_b2"][:L, None])
    shared["tz2_bs"] = np.ascontiguousarray(d["tz_b2"][L:, None])

    in_maps = []
    for c in range(N_CORES):
        m = dict(shared)
        m["x_rev"] = np.ascontiguousarray(x_rev[:, :, c * B:(c + 1) * B])
        in_maps.append(m)
    return in_maps


def kernel(**inputs):
    from concourse.bass_utils import run_bass_kernel_spmd

    obs = np.asarray(inputs["obs_tps"], np.float32)[:N_TP]
    _, vids = _dt_variants(obs, N_TP)
    key = (N_TP, SP_ITERS, vids)
    if key not in _CACHE:
        _CACHE[key] = _build(N_TP, SP_ITERS, vids)
    nc = _CACHE[key]

    in_maps = _prep_in_maps(inputs, N_TP)
    res = run_bass_kernel_spmd(nc, in_maps, list(range(N_CORES)))
    outs = res.results

    mean = np.empty((1, N_SUBJ, L), np.float32)
    std = np.empty((1, N_SUBJ, L), np.float32)
    for c in range(N_CORES):
        mean[0, c * B:(c + 1) * B] = outs[c]["out_m"].T
        std[0, c * B:(c + 1) * B] = outs[c]["out_s"].T
    return mean, std


# revision 62
# speedup vs baseline: 1.0391x; 1.0003x over previous
"""Trainium2 Bass kernel for an ODE-RNN encoder (z0 posterior).

Model: 128-step reversed-time GRU-like recurrence with an Euler ODE step on
the mean channel, then a final transform producing (mean_z0, std_z0).

Strategy: data-parallel over the subject (batch) dim across 8 NeuronCores,
weights replicated.  Everything runs on-chip in a transposed layout
([feature, batch], batch=256 on the free dim).  Key points vs a naive port:
- matmul operands and the recurrent state are bf16 (fp32 PSUM accumulate).
- biases ride a ones-row appended to the streamed x tile (layer-1 nets),
  ACT per-partition bias vectors, or K=1 matmuls; zero per-step bias ops.
- the Euler step is folded into the ode2 weights: dt takes very few distinct
  values over the scan, so dt*ode_w2 / dt*ode_b2 are pre-baked per distinct
  value and Yode = Ym + psum(ode2_dt) is a single DVE add (no ACT hop).
- the reset-gate application r.Y = 0.5(1+tanh(zR/2)).Y uses 0.5-pre-scaled
  ns1 weights and am2 = (1+T).Yode as one scalar_tensor_tensor op, so ns1
  costs only K=66 (x) + 2x K=128 (state) matmuls.
- the observation mask m is computed on the host, packed as a 0.5*m row in
  the x stream, and broadcast across partitions with a K=1 matmul; the gate
  factor  -G = (tanh(zU/2) - 1)*(0.5 m)  is one STT op and the blends use
  `subtract` to absorb the sign.
- sigmoid(z) = 0.5 + 0.5*tanh(z/2) keeps every transcendental in the
  resident `exp_and_others` ACT table set (no per-step table switches).
- softplus(x) = log1p(exp(x)) via one Newton step: with w = exp(-|x|) and
  seed y0 = relu(x) + ln2*w, the correction (1+e^x)e^{-y0} collapses
  EXACTLY to (1+w)*2^{-w}, so the tail is 3 ACT ops (Abs, Exp, Exp) and a
  short DVE chain (~1.8e-3 max abs err, under the bf16 noise floor).
- TRN2 allows ONE sync wait per instruction; Bacc legalizes the rest, but
  K=1 dummy matmuls + accumulation-group ordering keep the PE free of
  multi-wait event-semaphore preambles in the steady state.
"""
import sys
import numpy as np
import ml_dtypes

for _p in ("/opt/trn_rl_repo", "/root/.axon_site/_ro/trn_rl_repo"):
    if _p not in sys.path:
        sys.path.append(_p)

N_SUBJ, N_TP, INPUT_DIM, LATENT, N_UNIT = 2048, 128, 64, 128, 256
HALF = INPUT_DIM // 2
N_CORES = 8
B = N_SUBJ // N_CORES          # 256 subjects per core (free dim)
L = LATENT
SP_ITERS = 1                   # kept for test.py compat (cache key)
LN2 = float(np.log(2.0))
BF = ml_dtypes.bfloat16

_CACHE = {}


# --------------------------------------------------------------------------
# Bass program
# --------------------------------------------------------------------------
def _build(n_tp, sp_iters, vids):
    """vids: per-step index into the distinct-dt weight variants."""
    import concourse.mybir as mybir
    from concourse import bacc, tile

    F32 = mybir.dt.float32
    B16 = mybir.dt.bfloat16
    FP8 = mybir.dt.float8e4
    DR = mybir.MatmulPerfMode.DoubleRow
    AF = mybir.ActivationFunctionType
    OP = mybir.AluOpType

    n_var = max(vids) + 1

    # Bacc (not plain Bass): its compile() legalizes the TRN2 one-sync-wait-
    # per-instruction limit (event-semaphore splitting, matmul-wait moves).
    nc = bacc.Bacc(None)

    # ---- DRAM I/O ----
    # x slab per step: row 0 = 0.5*mask-observed, rows 1..64 data
    d_x = nc.dram_tensor("x_rev", [n_tp, INPUT_DIM + 1, B], B16,
                         kind="ExternalInput")

    bspec = {  # bf16 weights (matmul operands)
        "ug1_k0": [L, N_UNIT], "ug1_k1": [L, N_UNIT], "ug1_kx": [INPUT_DIM + 2, N_UNIT],
        "rg1_k0": [L, N_UNIT], "rg1_k1": [L, N_UNIT], "rg1_kx": [INPUT_DIM + 2, N_UNIT],
        "ns1_k0": [L, N_UNIT], "ns1_k1": [L, N_UNIT], "ns1_kx": [INPUT_DIM + 2, N_UNIT],
        "ode1_w": [L, N_UNIT],
        "ns2_k0": [128, 2 * L], "ns2_k1": [128, 2 * L], "ns2_bm16": [1, L],
        "neg_eye": [L, L],
        "tz1_k0": [L, N_UNIT], "tz1_k1": [L, N_UNIT], "tz1_b": [1, N_UNIT],
        "tz2_k0": [128, 2 * L], "tz2_k1": [128, 2 * L],
    }
    # fp8 DoubleRow weights: K=256 reductions in one PE instruction (2 rows
    # per cycle).  Host-validated: fp8 on the gate layer-2 and the dt-baked
    # ode layer-2 keeps end-to-end error ~9e-3 (budget 2e-2); ns1/ns2 in fp8
    # would blow it.
    f8spec = {"ug2_k01": [128, 2, L], "rg2_k01": [128, 2, L]}
    for u in range(n_var):  # dt-baked ode layer-2 weights + dt*b2 rows
        f8spec[f"o2k01_{u}"] = [128, 2, L]
        bspec[f"o2b_{u}"] = [1, L]
    fspec = {  # fp32 per-partition columns (ACT bias vectors)
        "ode1_bc": [128, 2], "ug2_bc": [128, 1], "rg2_bc": [128, 1],
        "ns2_bs": [128, 1], "tz2_bm": [128, 1], "tz2_bs": [128, 1],
    }
    d_w = {k: nc.dram_tensor(k, v, B16, kind="ExternalInput") for k, v in bspec.items()}
    d_w.update({k: nc.dram_tensor(k, v, FP8, kind="ExternalInput")
                for k, v in f8spec.items()})
    d_w.update({k: nc.dram_tensor(k, v, F32, kind="ExternalInput")
                for k, v in fspec.items()})

    d_om = nc.dram_tensor("out_m", [L, B], F32, kind="ExternalOutput")
    d_os = nc.dram_tensor("out_s", [L, B], F32, kind="ExternalOutput")

    with tile.TileContext(nc) as tc:
        with (
            tc.tile_pool(name="const", bufs=1) as cp,
            tc.tile_pool(name="work", bufs=3) as wp,
            tc.tile_pool(name="ps", bufs=1, space="PSUM") as pp,
        ):
            # ---- prologue: step-0's x slab and the memsets go FIRST so the
            # x0 DMA isn't queued behind ~1MB of weight DMAs; step 0 can
            # start as soon as its first weights land ----
            xbufs = []
            for j in range(3):
                xb = cp.tile([INPUT_DIM + 2, B], B16, tag=f"xb{j}", name=f"xb{j}")
                # rows 0..64 are DMA-overwritten each step; row 65 stays 1.0
                # (memset must start at a 32-aligned partition, so cover all)
                nc.vector.memset(xb[:], 1.0)
                xbufs.append(xb)
            nc.sync.dma_start(xbufs[0][:INPUT_DIM + 1, :], d_x[0])
            ones_row = cp.tile([1, B], B16, tag="ones_row", name="ones_row")
            nc.vector.memset(ones_row[:], 1.0)
            ones1 = cp.tile([1, 128], B16, tag="ones1", name="ones1")
            nc.vector.memset(ones1[:], 1.0)
            # state lives in bf16 (matmul-input rounding dominates anyway)
            ym = [cp.tile([L, B], B16, tag=f"ym{i}", name=f"ym{i}") for i in range(2)]
            ys = [cp.tile([L, B], B16, tag=f"ys{i}", name=f"ys{i}") for i in range(2)]
            nc.vector.memset(ym[0][:], 0.0)
            nc.vector.memset(ys[0][:], 0.0)

            # ---- resident constants / weights ----
            w = {}
            for k, shp in bspec.items():
                w[k] = cp.tile(shp, B16, tag=k, name=k)
                nc.sync.dma_start(w[k][:], d_w[k][:])
            for k, shp in f8spec.items():
                w[k] = cp.tile(shp, FP8, tag=k, name=k)
                nc.sync.dma_start(w[k][:], d_w[k][:])
            for k, shp in fspec.items():
                w[k] = cp.tile(shp, F32, tag=k, name=k)
                nc.sync.dma_start(w[k][:], d_w[k][:])

            mm = nc.tensor.matmul

            # Warm the PE's clock past every weight DMA with K=1 dummy
            # matmuls so steady-state matmuls only wait on one producer.
            scr = pp.tile([1, 16], F32, tag="scr", name="scr")
            for k in bspec:
                mm(scr[0:1, 0:1], w[k][0:1, 0:1], w[k][0:1, 1:2],
                   start=True, stop=True)
            for k in f8spec:
                mm(scr[0:1, 0:1], w[k][0:1, 0:1, 0:1], w[k][0:1, 0:1, 1:2],
                   start=True, stop=True)
            # DVE/ACT read fp32 DMA-produced columns: warm those clocks too
            nf = len(fspec)
            warm_dv = cp.tile([1, 2 * nf], F32, tag="warm_dv", name="warm_dv")
            for j, k in enumerate(fspec):
                nc.vector.tensor_copy(warm_dv[0:1, j:j + 1], w[k][0:1, 0:1])
                nc.scalar.copy(warm_dv[0:1, nf + j:nf + j + 1], w[k][0:1, 0:1])

            # ---- the recurrence ----
            from concourse.tile_rust import add_dep_helper
            cc = float(np.float32(1e-6) - np.float32(1.0))
            for t in range(n_tp):
                cur, nxt = t % 2, (t + 1) % 2
                xb = xbufs[t % 3]
                u = vids[t]
                if t + 1 < n_tp:  # prefetch next step's x slab
                    nc.sync.dma_start(xbufs[(t + 1) % 3][:INPUT_DIM + 1, :],
                                      d_x[t + 1])
                # x is prefetched a full step ahead, so the first kx matmul's
                # DMA wait is already satisfied in steady state: no dummy
                # absorber needed (saves one PE instruction per step)

                # One start=True per PSUM bank per step (it clears the whole
                # bank's has_written bits); every other matmul accumulates or
                # first-touch-overwrites per element, so groups can interleave
                # freely.  x-only matmuls go first: they are ready before the
                # previous step's state tail finishes, keeping the PE fed.
                psA = pp.tile([128, 4 * B], F32, tag="psA", name="psA")
                psC = pp.tile([128, 2 * B], F32, tag="psC", name="psC")
                psF = pp.tile([128, 2 * B], F32, tag="psF", name="psF")
                # dt*ode_b2 broadcast opens the psF bank (always-ready K=1)
                mm(psF[:, 0:B], w[f"o2b_{u}"][:], ones_row[:],
                   start=True, stop=False)
                # host-computed 0.5*mask row broadcast to all partitions
                mm(psF[:, B:], ones1[:], xb[0:1, :],
                   start=False, stop=False)
                for gi, net in ((1, "rg1"), (0, "ug1")):
                    for m in range(2):
                        sl = psA[:, (2 * gi + m) * B:(2 * gi + m + 1) * B]
                        ms = slice(m * 128, (m + 1) * 128)
                        mm(sl, w[net + "_kx"][:, ms], xb[:],
                           start=(m == 0), stop=False)
                for m in range(2):
                    ms = slice(m * 128, (m + 1) * 128)
                    mm(psC[:, m * B:(m + 1) * B], w["ns1_kx"][:, ms], xb[:],
                       start=(m == 0), stop=False)

                # ODE hidden: tanh(ode_w1^T @ Ym + b1), fp8 out; the whole
                # dt-baked K=256 ode layer-2 is ONE fp8 DoubleRow matmul
                psB = pp.tile([128, 2 * B], F32, tag="psB", name="psB")
                h_ode = wp.tile([128, 2, B], FP8, tag="h_ode", name="h_ode")
                for m in range(2):
                    sl = psB[:, m * B:(m + 1) * B]
                    ms = slice(m * 128, (m + 1) * 128)
                    mm(sl, w["ode1_w"][:, ms], ym[cur][:], start=(m == 0), stop=(m == 1))
                    nc.scalar.activation(h_ode[:, m:m + 1, :], sl, AF.Tanh,
                                         bias=w["ode1_bc"][:, m:m + 1])
                mm(psF[:, 0:B], w[f"o2k01_{u}"][:], h_ode[:],
                   start=False, stop=True, perf_mode=DR)

                # Yode = Ym + dt*(ode_out + b2), dt baked into o2k*/o2b:
                # a single DVE add off the PSUM accumulator
                yode = wp.tile([L, B], B16, tag="yode", name="yode")
                nc.vector.tensor_tensor(yode[:], psF[:, 0:B], ym[cur][:], op=OP.add)

                # gate layer 1 remaining k-tiles; yode (ready first) then ys,
                # per-group contiguous so each m-half completes as soon as its
                # last input lands and its tanh can fire.  The rg m0 pair is
                # pinned to run first: its tanh is on the critical std cycle,
                # and the scheduler otherwise queues rg_k1m0 behind ug work
                # (~1.7us of cycle per step).
                i_g1 = {}
                for gi, net in ((1, "rg1"), (0, "ug1")):
                    for m in range(2):
                        sl = psA[:, (2 * gi + m) * B:(2 * gi + m + 1) * B]
                        ms = slice(m * 128, (m + 1) * 128)
                        i_g1[net, m, 0] = mm(sl, w[net + "_k0"][:, ms], yode[:],
                                             start=False, stop=False)
                        i_g1[net, m, 1] = mm(sl, w[net + "_k1"][:, ms], ys[cur][:],
                                             start=False, stop=(m == 1))


                # layer 2 per gate; rg (reset gate) first: the critical
                # chain runs through R -> as2 -> ns1, U is only needed at
                # the final blend.  rg hidden tanh split per half for the
                # chain; ug hidden batched into one ACT op (off-chain).
                h_g1 = wp.tile([128, 4, B], FP8, tag="h_g1", name="h_g1")
                psD = pp.tile([128, 2 * B], F32, tag="psD", name="psD")
                t_ur = wp.tile([128, 2 * B], B16, tag="t_ur", name="t_ur")
                # one batched tanh: the DoubleRow rg2 matmul consumes both
                # halves at once, so splitting buys no early start and the
                # batch saves ~270ns of serial ACT on the cycle
                nc.scalar.activation(h_g1[:, 2:4, :], psA[:, 2 * B:4 * B],
                                     AF.Tanh)
                mm(psD[:, B:], w["rg2_k01"][:], h_g1[:, 2:4, :],
                   start=True, stop=False, perf_mode=DR)
                i_tur_r = nc.scalar.activation(t_ur[:, B:], psD[:, B:], AF.Tanh,
                                               bias=w["rg2_bc"][:, 0:1], scale=0.5)
                i_ugh = nc.scalar.activation(h_g1[:, 0:2, :], psA[:, 0:2 * B],
                                             AF.Tanh)
                add_dep_helper(i_ugh.ins, i_tur_r.ins, False, "rg ACT priority")
                mm(psD[:, 0:B], w["ug2_k01"][:], h_g1[:, 0:2, :],
                   start=False, stop=True, perf_mode=DR)
                i_tur_u = nc.scalar.activation(t_ur[:, 0:B], psD[:, 0:B], AF.Tanh,
                                               bias=w["ug2_bc"][:, 0:1], scale=0.5)

                # reset-gate products (ns1 k0/k1 pre-scaled 0.5, so
                # r.Y = 0.5(1+T).Y needs only (1+T).Y here); as2 first: the
                # std channel is the critical cycle
                as2 = wp.tile([L, B], B16, tag="as2", name="as2")
                nc.vector.scalar_tensor_tensor(
                    as2[:], t_ur[:, B:], 1.0, ys[cur][:], op0=OP.add, op1=OP.mult)
                am2 = wp.tile([L, B], B16, tag="am2", name="am2")
                nc.vector.scalar_tensor_tensor(
                    am2[:], t_ur[:, B:], 1.0, yode[:], op0=OP.add, op1=OP.mult)
                for m in range(2):
                    sl = psC[:, m * B:(m + 1) * B]
                    ms = slice(m * 128, (m + 1) * 128)
                    mm(sl, w["ns1_k1"][:, ms], as2[:], start=False, stop=False)
                    mm(sl, w["ns1_k0"][:, ms], am2[:], start=False, stop=(m == 1))

                # new-state layer 2: NM | NS pre-acts.  The NM half also
                # accumulates (+bm - Yode); nosync deps keep the bank's
                # start=True matmul first in the PE schedule.
                h_ns = wp.tile([128, 2 * B], B16, tag="h_ns", name="h_ns")
                psE = pp.tile([128, 2 * B], F32, tag="psE", name="psE")
                # bm (always ready) opens the bank and ne (yode-gated) joins
                # it early in the middle, so only the four h_ns-gated matmuls
                # remain between the tanh and the psE group close that
                # releases the tail's readers
                i_bm = mm(psE[:, 0:B], w["ns2_bm16"][:], ones_row[:],
                          start=True, stop=False)
                i_ne = mm(psE[:, 0:B], w["neg_eye"][:], yode[:],
                          start=False, stop=False)
                add_dep_helper(i_ne.ins, i_bm.ins, False, "bank-start order")
                nc.scalar.activation(h_ns[:], psC[:], AF.Tanh)
                i_k0s = mm(psE[:, B:], w["ns2_k0"][:, 128:], h_ns[:, 0:B],
                           start=False, stop=False)
                add_dep_helper(i_k0s.ins, i_ne.ins, False, "bank-start order")
                mm(psE[:, 0:B], w["ns2_k0"][:, 0:128], h_ns[:, 0:B],
                   start=False, stop=False)
                mm(psE[:, B:], w["ns2_k1"][:, 128:], h_ns[:, B:],
                   start=False, stop=False)
                mm(psE[:, 0:B], w["ns2_k1"][:, 0:128], h_ns[:, B:],
                   start=False, stop=True)

                # -G = (tanh(zU/2) - 1) * 0.5m   (one STT; sign absorbed by
                # `subtract` in the blends)
                g = wp.tile([L, B], F32, tag="g", name="g")
                nc.vector.scalar_tensor_tensor(
                    g[:], t_ur[:, 0:B], 1.0, psF[:, B:], op0=OP.subtract,
                    op1=OP.mult)

                # std tail: softplus(x)=log1p(e^x) via one Newton step.
                # w = exp(-|x|); sp = relu(x) + ln2*w - 1 + (1+w)*2^{-w}
                rl = wp.tile([L, B], F32, tag="rl", name="rl")
                nc.vector.tensor_scalar(rl[:], psE[:, B:], w["ns2_bs"][:, 0:1],
                                        0.0, op0=OP.add, op1=OP.max)

                # mean channel: Ym' = Yode - (-G)*(NM + bm - Yode)
                pm = wp.tile([L, B], F32, tag="pm", name="pm")
                nc.vector.tensor_tensor(pm[:], g[:], psE[:, 0:B], op=OP.mult)
                nc.vector.tensor_tensor(ym[nxt][:], yode[:], pm[:], op=OP.subtract)

                xa = wp.tile([L, B], F32, tag="xa", name="xa")
                nc.scalar.activation(xa[:], psE[:, B:], AF.Abs,
                                     bias=w["ns2_bs"][:, 0:1])
                wx = wp.tile([L, B], F32, tag="wx", name="wx")
                nc.scalar.activation(wx[:], xa[:], AF.Exp, scale=-1.0)
                vx = wp.tile([L, B], F32, tag="vx", name="vx")
                nc.scalar.activation(vx[:], wx[:], AF.Exp, scale=-LN2)
                h0 = wp.tile([L, B], F32, tag="h0", name="h0")
                nc.vector.scalar_tensor_tensor(
                    h0[:], wx[:], LN2, rl[:], op0=OP.mult, op1=OP.add)
                h1 = wp.tile([L, B], F32, tag="h1", name="h1")
                nc.vector.scalar_tensor_tensor(
                    h1[:], h0[:], cc, ys[cur][:], op0=OP.add, op1=OP.subtract)
                aw = wp.tile([L, B], F32, tag="aw", name="aw")
                nc.vector.scalar_tensor_tensor(
                    aw[:], wx[:], 1.0, vx[:], op0=OP.add, op1=OP.mult)
                h2 = wp.tile([L, B], F32, tag="h2", name="h2")
                nc.vector.tensor_tensor(h2[:], h1[:], aw[:], op=OP.add)
                p1 = wp.tile([L, B], F32, tag="p1", name="p1")
                nc.vector.tensor_tensor(p1[:], g[:], h2[:], op=OP.mult)
                nc.vector.tensor_tensor(ys[nxt][:], ys[cur][:], p1[:],
                                        op=OP.subtract)

            # ---- final transform ----
            fin = n_tp % 2
            psB = pp.tile([128, 2 * B], F32, tag="psB", name="psB")
            for m in range(2):
                sl = psB[:, m * B:(m + 1) * B]
                ms = slice(m * 128, (m + 1) * 128)
                mm(sl, w["tz1_b"][:, ms], ones_row[:], start=True, stop=False)
                mm(sl, w["tz1_k0"][:, ms], ym[fin][:], start=False, stop=False)
                mm(sl, w["tz1_k1"][:, ms], ys[fin][:], start=False, stop=True)
            h_tz = wp.tile([128, 2 * B], B16, tag="h_ode", name="h_tz")
            nc.scalar.activation(h_tz[:], psB[:], AF.Tanh)
            psE = pp.tile([128, 2 * B], F32, tag="psE", name="psE2")
            for m in range(2):
                sl = psE[:, m * B:(m + 1) * B]
                ms = slice(m * 128, (m + 1) * 128)
                mm(sl, w["tz2_k0"][:, ms], h_tz[:, 0:B], start=True, stop=False)
                mm(sl, w["tz2_k1"][:, ms], h_tz[:, B:], start=False, stop=True)
            o_m = wp.tile([L, B], F32, tag="o_m", name="o_m")
            nc.scalar.activation(o_m[:], psE[:, 0:B], AF.Identity,
                                 bias=w["tz2_bm"][:, 0:1])
            o_s = wp.tile([L, B], F32, tag="o_s", name="o_s")
            nc.scalar.activation(o_s[:], psE[:, B:], AF.Abs,
                                 bias=w["tz2_bs"][:, 0:1])
            nc.sync.dma_start(d_om[:], o_m[:])
            nc.sync.dma_start(d_os[:], o_s[:])

    nc.compile()
    return nc


# --------------------------------------------------------------------------
# host-side packing
# --------------------------------------------------------------------------
def _dt_variants(obs, n_tp):
    F = np.float32
    dd = (obs[:-1] - obs[1:])[::-1]
    dts = np.concatenate([np.full((1,), -0.01, F), dd]).astype(F)
    uniq, vids = np.unique(dts, return_inverse=True)
    return uniq, tuple(int(v) for v in vids)


def _prep_in_maps(inputs, n_tp):
    F = np.float32
    d = {k: np.ascontiguousarray(np.asarray(v, F)) for k, v in inputs.items()}
    obs = d["obs_tps"][:n_tp]
    data = d["data"][:, :n_tp]

    uniq, vids = _dt_variants(obs, n_tp)

    # x slab: [t, c, subj] reversed in time; row 0 = 0.5 * (any-observed)
    xr = data.transpose(1, 2, 0)[::-1]                    # [t, 64, subj]
    m_row = F(0.5) * (xr[:, HALF:].sum(axis=1, keepdims=True) > 0)  # [t,1,subj]
    x_rev = np.concatenate([m_row, xr], axis=1).astype(BF)  # [t, 65, subj]
    x_rev = np.ascontiguousarray(x_rev)

    ns_w1s = d["ns_w1"].copy()
    ns_w1s[:2 * L] *= F(0.5)

    def kx(w1, b1):
        # row 0 = 0 (mask row), rows 1..64 = x weights, row 65 = bias
        return np.vstack([np.zeros((1, w1.shape[1]), F), w1[2 * L:], b1[None, :]])

    bf = {
        "ug1_k0": d["ug_w1"][:L], "ug1_k1": d["ug_w1"][L:2 * L],
        "ug1_kx": kx(d["ug_w1"], d["ug_b1"]),
        "rg1_k0": d["rg_w1"][:L], "rg1_k1": d["rg_w1"][L:2 * L],
        "rg1_kx": kx(d["rg_w1"], d["rg_b1"]),
        "ns1_k0": ns_w1s[:L], "ns1_k1": ns_w1s[L:2 * L],
        "ns1_kx": kx(d["ns_w1"], d["ns_b1"]),
        "ode1_w": d["ode_w1"],
        "ns2_k0": d["ns_w2"][:128], "ns2_k1": d["ns_w2"][128:],
        "ns2_bm16": d["ns_b2"][None, :L],
        "neg_eye": -np.eye(L, dtype=F),
        "tz1_k0": d["tz_w1"][:L], "tz1_k1": d["tz_w1"][L:],
        "tz1_b": d["tz_b1"][None, :],
        "tz2_k0": d["tz_w2"][:128], "tz2_k1": d["tz_w2"][128:],
    }
    for u, dtv in enumerate(uniq):
        bf[f"o2b_{u}"] = d["ode_b2"][None, :] * dtv
    shared = {k: np.ascontiguousarray(v.astype(BF)) for k, v in bf.items()}
    # fp8 DoubleRow stationaries: [part, ktile, M] with ktile = (rows 0:128,
    # rows 128:256) of the K=256 layer-2 weights
    F8 = ml_dtypes.float8_e4m3fn

    def k01(w2):
        return np.ascontiguousarray(
            np.stack([w2[:128], w2[128:]], axis=1).astype(F8))

    shared["ug2_k01"] = k01(d["ug_w2"])
    shared["rg2_k01"] = k01(d["rg_w2"])
    for u, dtv in enumerate(uniq):
        shared[f"o2k01_{u}"] = k01(d["ode_w2"] * dtv)
    shared["ode1_bc"] = np.ascontiguousarray(d["ode_b1"].reshape(2, 128).T)
    shared["ug2_bc"] = np.ascontiguousarray(d["ug_b2"][:, None] * F(0.5))
    shared["rg2_bc"] = np.ascontiguousarray(d["rg_b2"][:, None] * F(0.5))
    shared["ns2_bs"] = np.ascontiguousarray(d["ns_b2"][L:, None])
    shared["tz2_bm"] = np.ascontiguousarray(d["tz_b2"][:L, None])
    shared["tz2_bs"] = np.ascontiguousarray(d["tz_b2"][L:, None])

    in_maps = []
    for c in range(N_CORES):
        m = dict(shared)
        m["x_rev"] = np.ascontiguousarray(x_rev[:, :, c * B:(c + 1) * B])
        in_maps.append(m)
    return in_maps


def kernel(**inputs):
    from concourse.bass_utils import run_bass_kernel_spmd

    obs = np.asarray(inputs["obs_tps"], np.float32)[:N_TP]
    _, vids = _dt_variants(obs, N_TP)
    key = (N_TP, SP_ITERS, vids)
    if key not in _CACHE:
        _CACHE[key] = _build(N_TP, SP_ITERS, vids)
    nc = _CACHE[key]

    in_maps = _prep_in_maps(inputs, N_TP)
    res = run_bass_kernel_spmd(nc, in_maps, list(range(N_CORES)))
    outs = res.results

    mean = np.empty((1, N_SUBJ, L), np.float32)
    std = np.empty((1, N_SUBJ, L), np.float32)
    for c in range(N_CORES):
        mean[0, c * B:(c + 1) * B] = outs[c]["out_m"].T
        std[0, c * B:(c + 1) * B] = outs[c]["out_s"].T
    return mean, std


# revision 63
# speedup vs baseline: 1.0408x; 1.0017x over previous
"""Trainium2 Bass kernel for an ODE-RNN encoder (z0 posterior).

Model: 128-step reversed-time GRU-like recurrence with an Euler ODE step on
the mean channel, then a final transform producing (mean_z0, std_z0).

Strategy: data-parallel over the subject (batch) dim across 8 NeuronCores,
weights replicated.  Everything runs on-chip in a transposed layout
([feature, batch], batch=256 on the free dim).  Key points vs a naive port:
- matmul operands and the recurrent state are bf16 (fp32 PSUM accumulate).
- biases ride a ones-row appended to the streamed x tile (layer-1 nets),
  ACT per-partition bias vectors, or K=1 matmuls; zero per-step bias ops.
- the Euler step is folded into the ode2 weights: dt takes very few distinct
  values over the scan, so dt*ode_w2 / dt*ode_b2 are pre-baked per distinct
  value and Yode = Ym + psum(ode2_dt) is a single DVE add (no ACT hop).
- the reset-gate application r.Y = 0.5(1+tanh(zR/2)).Y uses 0.5-pre-scaled
  ns1 weights and am2 = (1+T).Yode as one scalar_tensor_tensor op, so ns1
  costs only K=66 (x) + 2x K=128 (state) matmuls.
- the observation mask m is computed on the host, packed as a 0.5*m row in
  the x stream, and broadcast across partitions with a K=1 matmul; the gate
  factor  -G = (tanh(zU/2) - 1)*(0.5 m)  is one STT op and the blends use
  `subtract` to absorb the sign.
- sigmoid(z) = 0.5 + 0.5*tanh(z/2) keeps every transcendental in the
  resident `exp_and_others` ACT table set (no per-step table switches).
- softplus(x) = log1p(exp(x)) via one Newton step: with w = exp(-|x|) and
  seed y0 = relu(x) + ln2*w, the correction (1+e^x)e^{-y0} collapses
  EXACTLY to (1+w)*2^{-w}, so the tail is 3 ACT ops (Abs, Exp, Exp) and a
  short DVE chain (~1.8e-3 max abs err, under the bf16 noise floor).
- TRN2 allows ONE sync wait per instruction; Bacc legalizes the rest, but
  K=1 dummy matmuls + accumulation-group ordering keep the PE free of
  multi-wait event-semaphore preambles in the steady state.
"""
import sys
import numpy as np
import ml_dtypes

for _p in ("/opt/trn_rl_repo", "/root/.axon_site/_ro/trn_rl_repo"):
    if _p not in sys.path:
        sys.path.append(_p)

N_SUBJ, N_TP, INPUT_DIM, LATENT, N_UNIT = 2048, 128, 64, 128, 256
HALF = INPUT_DIM // 2
N_CORES = 8
B = N_SUBJ // N_CORES          # 256 subjects per core (free dim)
L = LATENT
SP_ITERS = 1                   # kept for test.py compat (cache key)
LN2 = float(np.log(2.0))
BF = ml_dtypes.bfloat16

_CACHE = {}


# --------------------------------------------------------------------------
# Bass program
# --------------------------------------------------------------------------
def _build(n_tp, sp_iters, vids):
    """vids: per-step index into the distinct-dt weight variants."""
    import concourse.mybir as mybir
    from concourse import bacc, tile

    F32 = mybir.dt.float32
    B16 = mybir.dt.bfloat16
    FP8 = mybir.dt.float8e4
    DR = mybir.MatmulPerfMode.DoubleRow
    AF = mybir.ActivationFunctionType
    OP = mybir.AluOpType

    n_var = max(vids) + 1

    # Bacc (not plain Bass): its compile() legalizes the TRN2 one-sync-wait-
    # per-instruction limit (event-semaphore splitting, matmul-wait moves).
    nc = bacc.Bacc(None)

    # ---- DRAM I/O ----
    # x slab per step: row 0 = 0.5*mask-observed, rows 1..64 data
    d_x = nc.dram_tensor("x_rev", [n_tp, INPUT_DIM + 1, B], B16,
                         kind="ExternalInput")

    # dict order = weight-DMA queue order = step-0 availability order: the
    # x/ode/gate weights step 0 needs in its first microseconds go first,
    # the tz weights (needed 1.6ms later, after the scan) go last
    bspec = {  # bf16 weights (matmul operands)
        "ug1_kx": [INPUT_DIM + 2, N_UNIT], "rg1_kx": [INPUT_DIM + 2, N_UNIT],
        "ns1_kx": [INPUT_DIM + 2, N_UNIT],
        "ode1_w": [L, N_UNIT],
        "ug1_k0": [L, N_UNIT], "ug1_k1": [L, N_UNIT],
        "rg1_k0": [L, N_UNIT], "rg1_k1": [L, N_UNIT],
        "ns1_k0": [L, N_UNIT], "ns1_k1": [L, N_UNIT],
        "ns2_k0": [128, 2 * L], "ns2_k1": [128, 2 * L], "ns2_bm16": [1, L],
        "neg_eye": [L, L],
    }
    for u in range(n_var):  # dt-baked ode layer-2 dt*b2 rows
        bspec[f"o2b_{u}"] = [1, L]
    bspec.update({
        "tz1_k0": [L, N_UNIT], "tz1_k1": [L, N_UNIT], "tz1_b": [1, N_UNIT],
        "tz2_k0": [128, 2 * L], "tz2_k1": [128, 2 * L],
    })
    # fp8 DoubleRow weights: K=256 reductions in one PE instruction (2 rows
    # per cycle).  Host-validated: fp8 on the gate layer-2 and the dt-baked
    # ode layer-2 keeps end-to-end error ~9e-3 (budget 2e-2); ns1/ns2 in fp8
    # would blow it.
    f8spec = {}
    for u in range(n_var):  # dt-baked ode layer-2 weights
        f8spec[f"o2k01_{u}"] = [128, 2, L]
    f8spec.update({"rg2_k01": [128, 2, L], "ug2_k01": [128, 2, L]})
    fspec = {  # fp32 per-partition columns (ACT bias vectors)
        "ode1_bc": [128, 2], "ug2_bc": [128, 1], "rg2_bc": [128, 1],
        "ns2_bs": [128, 1], "tz2_bm": [128, 1], "tz2_bs": [128, 1],
    }
    d_w = {k: nc.dram_tensor(k, v, B16, kind="ExternalInput") for k, v in bspec.items()}
    d_w.update({k: nc.dram_tensor(k, v, FP8, kind="ExternalInput")
                for k, v in f8spec.items()})
    d_w.update({k: nc.dram_tensor(k, v, F32, kind="ExternalInput")
                for k, v in fspec.items()})

    d_om = nc.dram_tensor("out_m", [L, B], F32, kind="ExternalOutput")
    d_os = nc.dram_tensor("out_s", [L, B], F32, kind="ExternalOutput")

    with tile.TileContext(nc) as tc:
        with (
            tc.tile_pool(name="const", bufs=1) as cp,
            tc.tile_pool(name="work", bufs=3) as wp,
            tc.tile_pool(name="ps", bufs=1, space="PSUM") as pp,
        ):
            # ---- prologue: step-0's x slab and the memsets go FIRST so the
            # x0 DMA isn't queued behind ~1MB of weight DMAs; step 0 can
            # start as soon as its first weights land ----
            xbufs = []
            for j in range(3):
                xb = cp.tile([INPUT_DIM + 2, B], B16, tag=f"xb{j}", name=f"xb{j}")
                # rows 0..64 are DMA-overwritten each step; row 65 stays 1.0
                # (memset must start at a 32-aligned partition, so cover all)
                nc.vector.memset(xb[:], 1.0)
                xbufs.append(xb)
            nc.sync.dma_start(xbufs[0][:INPUT_DIM + 1, :], d_x[0])
            ones_row = cp.tile([1, B], B16, tag="ones_row", name="ones_row")
            nc.vector.memset(ones_row[:], 1.0)
            ones1 = cp.tile([1, 128], B16, tag="ones1", name="ones1")
            nc.vector.memset(ones1[:], 1.0)
            # state lives in bf16 (matmul-input rounding dominates anyway)
            ym = [cp.tile([L, B], B16, tag=f"ym{i}", name=f"ym{i}") for i in range(2)]
            ys = [cp.tile([L, B], B16, tag=f"ys{i}", name=f"ys{i}") for i in range(2)]
            nc.vector.memset(ym[0][:], 0.0)
            nc.vector.memset(ys[0][:], 0.0)

            # ---- resident constants / weights ----
            w = {}
            for k, shp in bspec.items():
                w[k] = cp.tile(shp, B16, tag=k, name=k)
                nc.sync.dma_start(w[k][:], d_w[k][:])
            for k, shp in f8spec.items():
                w[k] = cp.tile(shp, FP8, tag=k, name=k)
                nc.sync.dma_start(w[k][:], d_w[k][:])
            for k, shp in fspec.items():
                w[k] = cp.tile(shp, F32, tag=k, name=k)
                nc.sync.dma_start(w[k][:], d_w[k][:])

            mm = nc.tensor.matmul

            # Warm the PE's clock past every weight DMA with K=1 dummy
            # matmuls so steady-state matmuls only wait on one producer.
            scr = pp.tile([1, 16], F32, tag="scr", name="scr")
            for k in bspec:
                mm(scr[0:1, 0:1], w[k][0:1, 0:1], w[k][0:1, 1:2],
                   start=True, stop=True)
            for k in f8spec:
                mm(scr[0:1, 0:1], w[k][0:1, 0:1, 0:1], w[k][0:1, 0:1, 1:2],
                   start=True, stop=True)
            # DVE/ACT read fp32 DMA-produced columns: warm those clocks too
            nf = len(fspec)
            warm_dv = cp.tile([1, 2 * nf], F32, tag="warm_dv", name="warm_dv")
            for j, k in enumerate(fspec):
                nc.vector.tensor_copy(warm_dv[0:1, j:j + 1], w[k][0:1, 0:1])
                nc.scalar.copy(warm_dv[0:1, nf + j:nf + j + 1], w[k][0:1, 0:1])

            # ---- the recurrence ----
            from concourse.tile_rust import add_dep_helper
            cc = float(np.float32(1e-6) - np.float32(1.0))
            for t in range(n_tp):
                cur, nxt = t % 2, (t + 1) % 2
                xb = xbufs[t % 3]
                u = vids[t]
                if t + 1 < n_tp:  # prefetch next step's x slab
                    nc.sync.dma_start(xbufs[(t + 1) % 3][:INPUT_DIM + 1, :],
                                      d_x[t + 1])
                # x is prefetched a full step ahead, so the first kx matmul's
                # DMA wait is already satisfied in steady state: no dummy
                # absorber needed (saves one PE instruction per step)

                # One start=True per PSUM bank per step (it clears the whole
                # bank's has_written bits); every other matmul accumulates or
                # first-touch-overwrites per element, so groups can interleave
                # freely.  x-only matmuls go first: they are ready before the
                # previous step's state tail finishes, keeping the PE fed.
                psA = pp.tile([128, 4 * B], F32, tag="psA", name="psA")
                psC = pp.tile([128, 2 * B], F32, tag="psC", name="psC")
                psF = pp.tile([128, 2 * B], F32, tag="psF", name="psF")
                # dt*ode_b2 broadcast opens the psF bank (always-ready K=1)
                mm(psF[:, 0:B], w[f"o2b_{u}"][:], ones_row[:],
                   start=True, stop=False)
                # host-computed 0.5*mask row broadcast to all partitions
                mm(psF[:, B:], ones1[:], xb[0:1, :],
                   start=False, stop=False)
                for gi, net in ((1, "rg1"), (0, "ug1")):
                    for m in range(2):
                        sl = psA[:, (2 * gi + m) * B:(2 * gi + m + 1) * B]
                        ms = slice(m * 128, (m + 1) * 128)
                        mm(sl, w[net + "_kx"][:, ms], xb[:],
                           start=(m == 0), stop=False)
                for m in range(2):
                    ms = slice(m * 128, (m + 1) * 128)
                    mm(psC[:, m * B:(m + 1) * B], w["ns1_kx"][:, ms], xb[:],
                       start=(m == 0), stop=False)

                # ODE hidden: tanh(ode_w1^T @ Ym + b1), fp8 out; the whole
                # dt-baked K=256 ode layer-2 is ONE fp8 DoubleRow matmul
                psB = pp.tile([128, 2 * B], F32, tag="psB", name="psB")
                h_ode = wp.tile([128, 2, B], FP8, tag="h_ode", name="h_ode")
                for m in range(2):
                    sl = psB[:, m * B:(m + 1) * B]
                    ms = slice(m * 128, (m + 1) * 128)
                    mm(sl, w["ode1_w"][:, ms], ym[cur][:], start=(m == 0), stop=(m == 1))
                    nc.scalar.activation(h_ode[:, m:m + 1, :], sl, AF.Tanh,
                                         bias=w["ode1_bc"][:, m:m + 1])
                mm(psF[:, 0:B], w[f"o2k01_{u}"][:], h_ode[:],
                   start=False, stop=True, perf_mode=DR)

                # Yode = Ym + dt*(ode_out + b2), dt baked into o2k*/o2b:
                # a single DVE add off the PSUM accumulator
                yode = wp.tile([L, B], B16, tag="yode", name="yode")
                nc.vector.tensor_tensor(yode[:], psF[:, 0:B], ym[cur][:], op=OP.add)

                # gate layer 1 remaining k-tiles; yode (ready first) then ys,
                # per-group contiguous so each m-half completes as soon as its
                # last input lands and its tanh can fire.  The rg m0 pair is
                # pinned to run first: its tanh is on the critical std cycle,
                # and the scheduler otherwise queues rg_k1m0 behind ug work
                # (~1.7us of cycle per step).
                i_g1 = {}
                for gi, net in ((1, "rg1"), (0, "ug1")):
                    for m in range(2):
                        sl = psA[:, (2 * gi + m) * B:(2 * gi + m + 1) * B]
                        ms = slice(m * 128, (m + 1) * 128)
                        i_g1[net, m, 0] = mm(sl, w[net + "_k0"][:, ms], yode[:],
                                             start=False, stop=False)
                        i_g1[net, m, 1] = mm(sl, w[net + "_k1"][:, ms], ys[cur][:],
                                             start=False, stop=(m == 1))


                # layer 2 per gate; rg (reset gate) first: the critical
                # chain runs through R -> as2 -> ns1, U is only needed at
                # the final blend.  rg hidden tanh split per half for the
                # chain; ug hidden batched into one ACT op (off-chain).
                h_g1 = wp.tile([128, 4, B], FP8, tag="h_g1", name="h_g1")
                psD = pp.tile([128, 2 * B], F32, tag="psD", name="psD")
                t_ur = wp.tile([128, 2 * B], B16, tag="t_ur", name="t_ur")
                # one batched tanh: the DoubleRow rg2 matmul consumes both
                # halves at once, so splitting buys no early start and the
                # batch saves ~270ns of serial ACT on the cycle
                nc.scalar.activation(h_g1[:, 2:4, :], psA[:, 2 * B:4 * B],
                                     AF.Tanh)
                mm(psD[:, B:], w["rg2_k01"][:], h_g1[:, 2:4, :],
                   start=True, stop=False, perf_mode=DR)
                i_tur_r = nc.scalar.activation(t_ur[:, B:], psD[:, B:], AF.Tanh,
                                               bias=w["rg2_bc"][:, 0:1], scale=0.5)
                i_ugh = nc.scalar.activation(h_g1[:, 0:2, :], psA[:, 0:2 * B],
                                             AF.Tanh)
                add_dep_helper(i_ugh.ins, i_tur_r.ins, False, "rg ACT priority")
                mm(psD[:, 0:B], w["ug2_k01"][:], h_g1[:, 0:2, :],
                   start=False, stop=True, perf_mode=DR)
                i_tur_u = nc.scalar.activation(t_ur[:, 0:B], psD[:, 0:B], AF.Tanh,
                                               bias=w["ug2_bc"][:, 0:1], scale=0.5)

                # reset-gate products (ns1 k0/k1 pre-scaled 0.5, so
                # r.Y = 0.5(1+T).Y needs only (1+T).Y here); as2 first: the
                # std channel is the critical cycle
                as2 = wp.tile([L, B], B16, tag="as2", name="as2")
                nc.vector.scalar_tensor_tensor(
                    as2[:], t_ur[:, B:], 1.0, ys[cur][:], op0=OP.add, op1=OP.mult)
                am2 = wp.tile([L, B], B16, tag="am2", name="am2")
                nc.vector.scalar_tensor_tensor(
                    am2[:], t_ur[:, B:], 1.0, yode[:], op0=OP.add, op1=OP.mult)
                for m in range(2):
                    sl = psC[:, m * B:(m + 1) * B]
                    ms = slice(m * 128, (m + 1) * 128)
                    mm(sl, w["ns1_k1"][:, ms], as2[:], start=False, stop=False)
                    mm(sl, w["ns1_k0"][:, ms], am2[:], start=False, stop=(m == 1))

                # new-state layer 2: NM | NS pre-acts.  The NM half also
                # accumulates (+bm - Yode); nosync deps keep the bank's
                # start=True matmul first in the PE schedule.
                h_ns = wp.tile([128, 2 * B], B16, tag="h_ns", name="h_ns")
                psE = pp.tile([128, 2 * B], F32, tag="psE", name="psE")
                # bm (always ready) opens the bank and ne (yode-gated) joins
                # it early in the middle, so only the four h_ns-gated matmuls
                # remain between the tanh and the psE group close that
                # releases the tail's readers
                i_bm = mm(psE[:, 0:B], w["ns2_bm16"][:], ones_row[:],
                          start=True, stop=False)
                i_ne = mm(psE[:, 0:B], w["neg_eye"][:], yode[:],
                          start=False, stop=False)
                add_dep_helper(i_ne.ins, i_bm.ins, False, "bank-start order")
                nc.scalar.activation(h_ns[:], psC[:], AF.Tanh)
                i_k0s = mm(psE[:, B:], w["ns2_k0"][:, 128:], h_ns[:, 0:B],
                           start=False, stop=False)
                add_dep_helper(i_k0s.ins, i_ne.ins, False, "bank-start order")
                mm(psE[:, 0:B], w["ns2_k0"][:, 0:128], h_ns[:, 0:B],
                   start=False, stop=False)
                mm(psE[:, B:], w["ns2_k1"][:, 128:], h_ns[:, B:],
                   start=False, stop=False)
                mm(psE[:, 0:B], w["ns2_k1"][:, 0:128], h_ns[:, B:],
                   start=False, stop=True)

                # -G = (tanh(zU/2) - 1) * 0.5m   (one STT; sign absorbed by
                # `subtract` in the blends)
                g = wp.tile([L, B], F32, tag="g", name="g")
                nc.vector.scalar_tensor_tensor(
                    g[:], t_ur[:, 0:B], 1.0, psF[:, B:], op0=OP.subtract,
                    op1=OP.mult)

                # std tail: softplus(x)=log1p(e^x) via one Newton step.
                # w = exp(-|x|); sp = relu(x) + ln2*w - 1 + (1+w)*2^{-w}
                rl = wp.tile([L, B], F32, tag="rl", name="rl")
                nc.vector.tensor_scalar(rl[:], psE[:, B:], w["ns2_bs"][:, 0:1],
                                        0.0, op0=OP.add, op1=OP.max)

                # mean channel: Ym' = Yode - (-G)*(NM + bm - Yode)
                pm = wp.tile([L, B], F32, tag="pm", name="pm")
                nc.vector.tensor_tensor(pm[:], g[:], psE[:, 0:B], op=OP.mult)
                nc.vector.tensor_tensor(ym[nxt][:], yode[:], pm[:], op=OP.subtract)

                xa = wp.tile([L, B], F32, tag="xa", name="xa")
                nc.scalar.activation(xa[:], psE[:, B:], AF.Abs,
                                     bias=w["ns2_bs"][:, 0:1])
                wx = wp.tile([L, B], F32, tag="wx", name="wx")
                nc.scalar.activation(wx[:], xa[:], AF.Exp, scale=-1.0)
                vx = wp.tile([L, B], F32, tag="vx", name="vx")
                nc.scalar.activation(vx[:], wx[:], AF.Exp, scale=-LN2)
                h0 = wp.tile([L, B], F32, tag="h0", name="h0")
                nc.vector.scalar_tensor_tensor(
                    h0[:], wx[:], LN2, rl[:], op0=OP.mult, op1=OP.add)
                h1 = wp.tile([L, B], F32, tag="h1", name="h1")
                nc.vector.scalar_tensor_tensor(
                    h1[:], h0[:], cc, ys[cur][:], op0=OP.add, op1=OP.subtract)
                aw = wp.tile([L, B], F32, tag="aw", name="aw")
                nc.vector.scalar_tensor_tensor(
                    aw[:], wx[:], 1.0, vx[:], op0=OP.add, op1=OP.mult)
                h2 = wp.tile([L, B], F32, tag="h2", name="h2")
                nc.vector.tensor_tensor(h2[:], h1[:], aw[:], op=OP.add)
                p1 = wp.tile([L, B], F32, tag="p1", name="p1")
                nc.vector.tensor_tensor(p1[:], g[:], h2[:], op=OP.mult)
                nc.vector.tensor_tensor(ys[nxt][:], ys[cur][:], p1[:],
                                        op=OP.subtract)

            # ---- final transform ----
            fin = n_tp % 2
            psB = pp.tile([128, 2 * B], F32, tag="psB", name="psB")
            for m in range(2):
                sl = psB[:, m * B:(m + 1) * B]
                ms = slice(m * 128, (m + 1) * 128)
                mm(sl, w["tz1_b"][:, ms], ones_row[:], start=True, stop=False)
                mm(sl, w["tz1_k0"][:, ms], ym[fin][:], start=False, stop=False)
                mm(sl, w["tz1_k1"][:, ms], ys[fin][:], start=False, stop=True)
            h_tz = wp.tile([128, 2 * B], B16, tag="h_ode", name="h_tz")
            nc.scalar.activation(h_tz[:], psB[:], AF.Tanh)
            psE = pp.tile([128, 2 * B], F32, tag="psE", name="psE2")
            for m in range(2):
                sl = psE[:, m * B:(m + 1) * B]
                ms = slice(m * 128, (m + 1) * 128)
                mm(sl, w["tz2_k0"][:, ms], h_tz[:, 0:B], start=True, stop=False)
                mm(sl, w["tz2_k1"][:, ms], h_tz[:, B:], start=False, stop=True)
            o_m = wp.tile([L, B], F32, tag="o_m", name="o_m")
            nc.scalar.activation(o_m[:], psE[:, 0:B], AF.Identity,
                                 bias=w["tz2_bm"][:, 0:1])
            o_s = wp.tile([L, B], F32, tag="o_s", name="o_s")
            nc.scalar.activation(o_s[:], psE[:, B:], AF.Abs,
                                 bias=w["tz2_bs"][:, 0:1])
            nc.sync.dma_start(d_om[:], o_m[:])
            nc.sync.dma_start(d_os[:], o_s[:])

    nc.compile()
    return nc


# --------------------------------------------------------------------------
# host-side packing
# --------------------------------------------------------------------------
def _dt_variants(obs, n_tp):
    F = np.float32
    dd = (obs[:-1] - obs[1:])[::-1]
    dts = np.concatenate([np.full((1,), -0.01, F), dd]).astype(F)
    uniq, vids = np.unique(dts, return_inverse=True)
    return uniq, tuple(int(v) for v in vids)


def _prep_in_maps(inputs, n_tp):
    F = np.float32
    d = {k: np.ascontiguousarray(np.asarray(v, F)) for k, v in inputs.items()}
    obs = d["obs_tps"][:n_tp]
    data = d["data"][:, :n_tp]

    uniq, vids = _dt_variants(obs, n_tp)

    # x slab: [t, c, subj] reversed in time; row 0 = 0.5 * (any-observed)
    xr = data.transpose(1, 2, 0)[::-1]                    # [t, 64, subj]
    m_row = F(0.5) * (xr[:, HALF:].sum(axis=1, keepdims=True) > 0)  # [t,1,subj]
    x_rev = np.concatenate([m_row, xr], axis=1).astype(BF)  # [t, 65, subj]
    x_rev = np.ascontiguousarray(x_rev)

    ns_w1s = d["ns_w1"].copy()
    ns_w1s[:2 * L] *= F(0.5)

    def kx(w1, b1):
        # row 0 = 0 (mask row), rows 1..64 = x weights, row 65 = bias
        return np.vstack([np.zeros((1, w1.shape[1]), F), w1[2 * L:], b1[None, :]])

    bf = {
        "ug1_k0": d["ug_w1"][:L], "ug1_k1": d["ug_w1"][L:2 * L],
        "ug1_kx": kx(d["ug_w1"], d["ug_b1"]),
        "rg1_k0": d["rg_w1"][:L], "rg1_k1": d["rg_w1"][L:2 * L],
        "rg1_kx": kx(d["rg_w1"], d["rg_b1"]),
        "ns1_k0": ns_w1s[:L], "ns1_k1": ns_w1s[L:2 * L],
        "ns1_kx": kx(d["ns_w1"], d["ns_b1"]),
        "ode1_w": d["ode_w1"],
        "ns2_k0": d["ns_w2"][:128], "ns2_k1": d["ns_w2"][128:],
        "ns2_bm16": d["ns_b2"][None, :L],
        "neg_eye": -np.eye(L, dtype=F),
        "tz1_k0": d["tz_w1"][:L], "tz1_k1": d["tz_w1"][L:],
        "tz1_b": d["tz_b1"][None, :],
        "tz2_k0": d["tz_w2"][:128], "tz2_k1": d["tz_w2"][128:],
    }
    for u, dtv in enumerate(uniq):
        bf[f"o2b_{u}"] = d["ode_b2"][None, :] * dtv
    shared = {k: np.ascontiguousarray(v.astype(BF)) for k, v in bf.items()}
    # fp8 DoubleRow stationaries: [part, ktile, M] with ktile = (rows 0:128,
    # rows 128:256) of the K=256 layer-2 weights
    F8 = ml_dtypes.float8_e4m3fn

    def k01(w2):
        return np.ascontiguousarray(
            np.stack([w2[:128], w2[128:]], axis=1).astype(F8))

    shared["ug2_k01"] = k01(d["ug_w2"])
    shared["rg2_k01"] = k01(d["rg_w2"])
    for u, dtv in enumerate(uniq):
        shared[f"o2k01_{u}"] = k01(d["ode_w2"] * dtv)
    shared["ode1_bc"] = np.ascontiguousarray(d["ode_b1"].reshape(2, 128).T)
    shared["ug2_bc"] = np.ascontiguousarray(d["ug_b2"][:, None] * F(0.5))
    shared["rg2_bc"] = np.ascontiguousarray(d["rg_b2"][:, None] * F(0.5))
    shared["ns2_bs"] = np.ascontiguousarray(d["ns_b2"][L:, None])
    shared["tz2_bm"] = np.ascontiguousarray(d["tz_b2"][:L, None])
    shared["tz2_bs"] = np.ascontiguousarray(d["tz_b2"][L:, None])

    in_maps = []
    for c in range(N_CORES):
        m = dict(shared)
        m["x_rev"] = np.ascontiguousarray(x_rev[:, :, c * B:(c + 1) * B])
        in_maps.append(m)
    return in_maps


def kernel(**inputs):
    from concourse.bass_utils import run_bass_kernel_spmd

    obs = np.asarray(inputs["obs_tps"], np.float32)[:N_TP]
    _, vids = _dt_variants(obs, N_TP)
    key = (N_TP, SP_ITERS, vids)
    if key not in _CACHE:
        _CACHE[key] = _build(N_TP, SP_ITERS, vids)
    nc = _CACHE[key]

    in_maps = _prep_in_maps(inputs, N_TP)
    res = run_bass_kernel_spmd(nc, in_maps, list(range(N_CORES)))
    outs = res.results

    mean = np.empty((1, N_SUBJ, L), np.float32)
    std = np.empty((1, N_SUBJ, L), np.float32)
    for c in range(N_CORES):
        mean[0, c * B:(c + 1) * B] = outs[c]["out_m"].T
        std[0, c * B:(c + 1) * B] = outs[c]["out_s"].T
    return mean, std


# revision 65
# speedup vs baseline: 1.0409x; 1.0001x over previous
"""Trainium2 Bass kernel for an ODE-RNN encoder (z0 posterior).

Model: 128-step reversed-time GRU-like recurrence with an Euler ODE step on
the mean channel, then a final transform producing (mean_z0, std_z0).

Strategy: data-parallel over the subject (batch) dim across 8 NeuronCores,
weights replicated.  Everything runs on-chip in a transposed layout
([feature, batch], batch=256 on the free dim).  Key points vs a naive port:
- matmul operands and the recurrent state are bf16 (fp32 PSUM accumulate).
- biases ride a ones-row appended to the streamed x tile (layer-1 nets),
  ACT per-partition bias vectors, or K=1 matmuls; zero per-step bias ops.
- the Euler step is folded into the ode2 weights: dt takes very few distinct
  values over the scan, so dt*ode_w2 / dt*ode_b2 are pre-baked per distinct
  value and Yode = Ym + psum(ode2_dt) is a single DVE add (no ACT hop).
- the reset-gate application r.Y = 0.5(1+tanh(zR/2)).Y uses 0.5-pre-scaled
  ns1 weights and am2 = (1+T).Yode as one scalar_tensor_tensor op, so ns1
  costs only K=66 (x) + 2x K=128 (state) matmuls.
- the observation mask m is computed on the host, packed as a 0.5*m row in
  the x stream, and broadcast across partitions with a K=1 matmul; the gate
  factor  -G = (tanh(zU/2) - 1)*(0.5 m)  is one STT op and the blends use
  `subtract` to absorb the sign.
- sigmoid(z) = 0.5 + 0.5*tanh(z/2) keeps every transcendental in the
  resident `exp_and_others` ACT table set (no per-step table switches).
- softplus(x) = log1p(exp(x)) via one Newton step: with w = exp(-|x|) and
  seed y0 = relu(x) + ln2*w, the correction (1+e^x)e^{-y0} collapses
  EXACTLY to (1+w)*2^{-w}, so the tail is 3 ACT ops (Abs, Exp, Exp) and a
  short DVE chain (~1.8e-3 max abs err, under the bf16 noise floor).
- TRN2 allows ONE sync wait per instruction; Bacc legalizes the rest, but
  K=1 dummy matmuls + accumulation-group ordering keep the PE free of
  multi-wait event-semaphore preambles in the steady state.
"""
import sys
import numpy as np
import ml_dtypes

for _p in ("/opt/trn_rl_repo", "/root/.axon_site/_ro/trn_rl_repo"):
    if _p not in sys.path:
        sys.path.append(_p)

N_SUBJ, N_TP, INPUT_DIM, LATENT, N_UNIT = 2048, 128, 64, 128, 256
HALF = INPUT_DIM // 2
N_CORES = 8
B = N_SUBJ // N_CORES          # 256 subjects per core (free dim)
L = LATENT
SP_ITERS = 1                   # kept for test.py compat (cache key)
LN2 = float(np.log(2.0))
BF = ml_dtypes.bfloat16

_CACHE = {}


# --------------------------------------------------------------------------
# Bass program
# --------------------------------------------------------------------------
def _build(n_tp, sp_iters, vids):
    """vids: per-step index into the distinct-dt weight variants."""
    import concourse.mybir as mybir
    from concourse import bacc, tile

    F32 = mybir.dt.float32
    B16 = mybir.dt.bfloat16
    FP8 = mybir.dt.float8e4
    DR = mybir.MatmulPerfMode.DoubleRow
    AF = mybir.ActivationFunctionType
    OP = mybir.AluOpType

    n_var = max(vids) + 1

    # Bacc (not plain Bass): its compile() legalizes the TRN2 one-sync-wait-
    # per-instruction limit (event-semaphore splitting, matmul-wait moves).
    nc = bacc.Bacc(None)

    # ---- DRAM I/O ----
    # x slab per step: row 0 = 0.5*mask-observed, rows 1..64 data
    d_x = nc.dram_tensor("x_rev", [n_tp, INPUT_DIM + 1, B], B16,
                         kind="ExternalInput")

    # dict order = weight-DMA queue order = step-0 availability order: the
    # x/ode/gate weights step 0 needs in its first microseconds go first,
    # the tz weights (needed 1.6ms later, after the scan) go last
    bspec = {  # bf16 weights (matmul operands)
        "ug1_kx": [INPUT_DIM + 2, N_UNIT], "rg1_kx": [INPUT_DIM + 2, N_UNIT],
        "ns1_kx": [INPUT_DIM + 2, N_UNIT],
        "ode1_w": [L, N_UNIT],
        "ug1_k0": [L, N_UNIT], "ug1_k1": [L, N_UNIT],
        "rg1_k0": [L, N_UNIT], "rg1_k1": [L, N_UNIT],
        "ns1_k0": [L, N_UNIT], "ns1_k1": [L, N_UNIT],
        "ns2_k0": [128, 2 * L], "ns2_k1": [128, 2 * L], "ns2_bm16": [1, L],
        "neg_eye": [L, L],
    }
    for u in range(n_var):  # dt-baked ode layer-2 dt*b2 rows
        bspec[f"o2b_{u}"] = [1, L]
    bspec.update({
        "tz1_k0": [L, N_UNIT], "tz1_k1": [L, N_UNIT], "tz1_b": [1, N_UNIT],
        "tz2_k0": [128, 2 * L], "tz2_k1": [128, 2 * L],
    })
    # fp8 DoubleRow weights: K=256 reductions in one PE instruction (2 rows
    # per cycle).  Host-validated: fp8 on the gate layer-2 and the dt-baked
    # ode layer-2 keeps end-to-end error ~9e-3 (budget 2e-2); ns1/ns2 in fp8
    # would blow it.
    f8spec = {}
    for u in range(n_var):  # dt-baked ode layer-2 weights
        f8spec[f"o2k01_{u}"] = [128, 2, L]
    f8spec.update({"rg2_k01": [128, 2, L], "ug2_k01": [128, 2, L]})
    fspec = {  # fp32 per-partition columns (ACT bias vectors)
        "ode1_bc": [128, 2], "ug2_bc": [128, 1], "rg2_bc": [128, 1],
        "ns2_bs": [128, 1], "tz2_bm": [128, 1], "tz2_bs": [128, 1],
    }
    d_w = {k: nc.dram_tensor(k, v, B16, kind="ExternalInput") for k, v in bspec.items()}
    d_w.update({k: nc.dram_tensor(k, v, FP8, kind="ExternalInput")
                for k, v in f8spec.items()})
    d_w.update({k: nc.dram_tensor(k, v, F32, kind="ExternalInput")
                for k, v in fspec.items()})

    d_om = nc.dram_tensor("out_m", [L, B], F32, kind="ExternalOutput")
    d_os = nc.dram_tensor("out_s", [L, B], F32, kind="ExternalOutput")

    with tile.TileContext(nc) as tc:
        with (
            tc.tile_pool(name="const", bufs=1) as cp,
            tc.tile_pool(name="work", bufs=3) as wp,
            tc.tile_pool(name="ps", bufs=1, space="PSUM") as pp,
        ):
            # ---- prologue: step-0's x slab and the memsets go FIRST so the
            # x0 DMA isn't queued behind ~1MB of weight DMAs; step 0 can
            # start as soon as its first weights land ----
            xbufs = []
            for j in range(3):
                xb = cp.tile([INPUT_DIM + 2, B], B16, tag=f"xb{j}", name=f"xb{j}")
                # rows 0..64 are DMA-overwritten each step; row 65 stays 1.0
                # (memset must start at a 32-aligned partition, so cover all)
                nc.vector.memset(xb[:], 1.0)
                xbufs.append(xb)
            nc.sync.dma_start(xbufs[0][:INPUT_DIM + 1, :], d_x[0])
            ones_row = cp.tile([1, B], B16, tag="ones_row", name="ones_row")
            nc.vector.memset(ones_row[:], 1.0)
            ones1 = cp.tile([1, 128], B16, tag="ones1", name="ones1")
            nc.vector.memset(ones1[:], 1.0)
            # state lives in bf16 (matmul-input rounding dominates anyway)
            ym = [cp.tile([L, B], B16, tag=f"ym{i}", name=f"ym{i}") for i in range(2)]
            ys = [cp.tile([L, B], B16, tag=f"ys{i}", name=f"ys{i}") for i in range(2)]
            nc.vector.memset(ym[0][:], 0.0)
            nc.vector.memset(ys[0][:], 0.0)

            # ---- resident constants / weights ----
            # One merged DMA queue in first-use order: layer-1 x weights and
            # the fp8 ode layer-2 (step 0 needs them within microseconds)
            # lead; tz weights (first used after the whole scan) trail.
            w = {}

            def _load(names, spec, dt):
                for k in names:
                    w[k] = cp.tile(spec[k], dt, tag=k, name=k)
                    nc.sync.dma_start(w[k][:], d_w[k][:])

            _load(["ug1_kx", "rg1_kx", "ns1_kx", "ode1_w"], bspec, B16)
            _load([k for k in f8spec if k.startswith("o2k01")], f8spec, FP8)
            _load([k for k in bspec if not k.startswith("tz") and k not in w],
                  bspec, B16)
            _load([k for k in f8spec if k not in w], f8spec, FP8)
            _load(list(fspec), fspec, F32)
            _load([k for k in bspec if k.startswith("tz")], bspec, B16)

            mm = nc.tensor.matmul

            # Warm the PE's clock past every weight DMA with K=1 dummy
            # matmuls so steady-state matmuls only wait on one producer.
            # (tz weights get no warm-up: they are used once, 1.6ms after
            # their DMA lands, so the in-order PE stream must not wait for
            # their DMAs before step 0)
            scr = pp.tile([1, 16], F32, tag="scr", name="scr")
            for k in bspec:
                if k.startswith("tz"):
                    continue
                mm(scr[0:1, 0:1], w[k][0:1, 0:1], w[k][0:1, 1:2],
                   start=True, stop=True)
            for k in f8spec:
                mm(scr[0:1, 0:1], w[k][0:1, 0:1, 0:1], w[k][0:1, 0:1, 1:2],
                   start=True, stop=True)
            # DVE/ACT read fp32 DMA-produced columns: warm those clocks too
            nf = len(fspec)
            warm_dv = cp.tile([1, 2 * nf], F32, tag="warm_dv", name="warm_dv")
            for j, k in enumerate(fspec):
                nc.vector.tensor_copy(warm_dv[0:1, j:j + 1], w[k][0:1, 0:1])
                nc.scalar.copy(warm_dv[0:1, nf + j:nf + j + 1], w[k][0:1, 0:1])

            # ---- the recurrence ----
            from concourse.tile_rust import add_dep_helper
            cc = float(np.float32(1e-6) - np.float32(1.0))
            for t in range(n_tp):
                cur, nxt = t % 2, (t + 1) % 2
                xb = xbufs[t % 3]
                u = vids[t]
                if t + 1 < n_tp:  # prefetch next step's x slab
                    nc.sync.dma_start(xbufs[(t + 1) % 3][:INPUT_DIM + 1, :],
                                      d_x[t + 1])
                # x is prefetched a full step ahead, so the first kx matmul's
                # DMA wait is already satisfied in steady state: no dummy
                # absorber needed (saves one PE instruction per step)

                # One start=True per PSUM bank per step (it clears the whole
                # bank's has_written bits); every other matmul accumulates or
                # first-touch-overwrites per element, so groups can interleave
                # freely.  x-only matmuls go first: they are ready before the
                # previous step's state tail finishes, keeping the PE fed.
                psA = pp.tile([128, 4 * B], F32, tag="psA", name="psA")
                psC = pp.tile([128, 2 * B], F32, tag="psC", name="psC")
                psF = pp.tile([128, 2 * B], F32, tag="psF", name="psF")
                # dt*ode_b2 broadcast opens the psF bank (always-ready K=1)
                mm(psF[:, 0:B], w[f"o2b_{u}"][:], ones_row[:],
                   start=True, stop=False)
                # host-computed 0.5*mask row broadcast to all partitions
                mm(psF[:, B:], ones1[:], xb[0:1, :],
                   start=False, stop=False)
                for gi, net in ((1, "rg1"), (0, "ug1")):
                    for m in range(2):
                        sl = psA[:, (2 * gi + m) * B:(2 * gi + m + 1) * B]
                        ms = slice(m * 128, (m + 1) * 128)
                        mm(sl, w[net + "_kx"][:, ms], xb[:],
                           start=(m == 0), stop=False)
                for m in range(2):
                    ms = slice(m * 128, (m + 1) * 128)
                    mm(psC[:, m * B:(m + 1) * B], w["ns1_kx"][:, ms], xb[:],
                       start=(m == 0), stop=False)

                # ODE hidden: tanh(ode_w1^T @ Ym + b1), fp8 out; the whole
                # dt-baked K=256 ode layer-2 is ONE fp8 DoubleRow matmul
                psB = pp.tile([128, 2 * B], F32, tag="psB", name="psB")
                h_ode = wp.tile([128, 2, B], FP8, tag="h_ode", name="h_ode")
                for m in range(2):
                    sl = psB[:, m * B:(m + 1) * B]
                    ms = slice(m * 128, (m + 1) * 128)
                    mm(sl, w["ode1_w"][:, ms], ym[cur][:], start=(m == 0), stop=(m == 1))
                    nc.scalar.activation(h_ode[:, m:m + 1, :], sl, AF.Tanh,
                                         bias=w["ode1_bc"][:, m:m + 1])
                mm(psF[:, 0:B], w[f"o2k01_{u}"][:], h_ode[:],
                   start=False, stop=True, perf_mode=DR)

                # Yode = Ym + dt*(ode_out + b2), dt baked into o2k*/o2b:
                # a single DVE add off the PSUM accumulator
                yode = wp.tile([L, B], B16, tag="yode", name="yode")
                nc.vector.tensor_tensor(yode[:], psF[:, 0:B], ym[cur][:], op=OP.add)

                # gate layer 1 remaining k-tiles; yode (ready first) then ys,
                # per-group contiguous so each m-half completes as soon as its
                # last input lands and its tanh can fire.  The rg m0 pair is
                # pinned to run first: its tanh is on the critical std cycle,
                # and the scheduler otherwise queues rg_k1m0 behind ug work
                # (~1.7us of cycle per step).
                i_g1 = {}
                for gi, net in ((1, "rg1"), (0, "ug1")):
                    for m in range(2):
                        sl = psA[:, (2 * gi + m) * B:(2 * gi + m + 1) * B]
                        ms = slice(m * 128, (m + 1) * 128)
                        i_g1[net, m, 0] = mm(sl, w[net + "_k0"][:, ms], yode[:],
                                             start=False, stop=False)
                        i_g1[net, m, 1] = mm(sl, w[net + "_k1"][:, ms], ys[cur][:],
                                             start=False, stop=(m == 1))


                # layer 2 per gate; rg (reset gate) first: the critical
                # chain runs through R -> as2 -> ns1, U is only needed at
                # the final blend.  rg hidden tanh split per half for the
                # chain; ug hidden batched into one ACT op (off-chain).
                h_g1 = wp.tile([128, 4, B], FP8, tag="h_g1", name="h_g1")
                psD = pp.tile([128, 2 * B], F32, tag="psD", name="psD")
                t_ur = wp.tile([128, 2 * B], B16, tag="t_ur", name="t_ur")
                # one batched tanh: the DoubleRow rg2 matmul consumes both
                # halves at once, so splitting buys no early start and the
                # batch saves ~270ns of serial ACT on the cycle
                nc.scalar.activation(h_g1[:, 2:4, :], psA[:, 2 * B:4 * B],
                                     AF.Tanh)
                mm(psD[:, B:], w["rg2_k01"][:], h_g1[:, 2:4, :],
                   start=True, stop=False, perf_mode=DR)
                i_tur_r = nc.scalar.activation(t_ur[:, B:], psD[:, B:], AF.Tanh,
                                               bias=w["rg2_bc"][:, 0:1], scale=0.5)
                i_ugh = nc.scalar.activation(h_g1[:, 0:2, :], psA[:, 0:2 * B],
                                             AF.Tanh)
                add_dep_helper(i_ugh.ins, i_tur_r.ins, False, "rg ACT priority")
                mm(psD[:, 0:B], w["ug2_k01"][:], h_g1[:, 0:2, :],
                   start=False, stop=True, perf_mode=DR)
                i_tur_u = nc.scalar.activation(t_ur[:, 0:B], psD[:, 0:B], AF.Tanh,
                                               bias=w["ug2_bc"][:, 0:1], scale=0.5)

                # reset-gate products (ns1 k0/k1 pre-scaled 0.5, so
                # r.Y = 0.5(1+T).Y needs only (1+T).Y here); as2 first: the
                # std channel is the critical cycle
                as2 = wp.tile([L, B], B16, tag="as2", name="as2")
                nc.vector.scalar_tensor_tensor(
                    as2[:], t_ur[:, B:], 1.0, ys[cur][:], op0=OP.add, op1=OP.mult)
                am2 = wp.tile([L, B], B16, tag="am2", name="am2")
                nc.vector.scalar_tensor_tensor(
                    am2[:], t_ur[:, B:], 1.0, yode[:], op0=OP.add, op1=OP.mult)
                for m in range(2):
                    sl = psC[:, m * B:(m + 1) * B]
                    ms = slice(m * 128, (m + 1) * 128)
                    mm(sl, w["ns1_k1"][:, ms], as2[:], start=False, stop=False)
                    mm(sl, w["ns1_k0"][:, ms], am2[:], start=False, stop=(m == 1))

                # new-state layer 2: NM | NS pre-acts.  The NM half also
                # accumulates (+bm - Yode); nosync deps keep the bank's
                # start=True matmul first in the PE schedule.
                h_ns = wp.tile([128, 2 * B], B16, tag="h_ns", name="h_ns")
                psE = pp.tile([128, 2 * B], F32, tag="psE", name="psE")
                # bm (always ready) opens the bank and ne (yode-gated) joins
                # it early in the middle, so only the four h_ns-gated matmuls
                # remain between the tanh and the psE group close that
                # releases the tail's readers
                i_bm = mm(psE[:, 0:B], w["ns2_bm16"][:], ones_row[:],
                          start=True, stop=False)
                i_ne = mm(psE[:, 0:B], w["neg_eye"][:], yode[:],
                          start=False, stop=False)
                add_dep_helper(i_ne.ins, i_bm.ins, False, "bank-start order")
                nc.scalar.activation(h_ns[:], psC[:], AF.Tanh)
                i_k0s = mm(psE[:, B:], w["ns2_k0"][:, 128:], h_ns[:, 0:B],
                           start=False, stop=False)
                add_dep_helper(i_k0s.ins, i_ne.ins, False, "bank-start order")
                mm(psE[:, 0:B], w["ns2_k0"][:, 0:128], h_ns[:, 0:B],
                   start=False, stop=False)
                mm(psE[:, B:], w["ns2_k1"][:, 128:], h_ns[:, B:],
                   start=False, stop=False)
                mm(psE[:, 0:B], w["ns2_k1"][:, 0:128], h_ns[:, B:],
                   start=False, stop=True)

                # -G = (tanh(zU/2) - 1) * 0.5m   (one STT; sign absorbed by
                # `subtract` in the blends)
                g = wp.tile([L, B], F32, tag="g", name="g")
                nc.vector.scalar_tensor_tensor(
                    g[:], t_ur[:, 0:B], 1.0, psF[:, B:], op0=OP.subtract,
                    op1=OP.mult)

                # std tail: softplus(x)=log1p(e^x) via one Newton step.
                # w = exp(-|x|); sp = relu(x) + ln2*w - 1 + (1+w)*2^{-w}
                rl = wp.tile([L, B], F32, tag="rl", name="rl")
                nc.vector.tensor_scalar(rl[:], psE[:, B:], w["ns2_bs"][:, 0:1],
                                        0.0, op0=OP.add, op1=OP.max)

                # mean channel: Ym' = Yode - (-G)*(NM + bm - Yode)
                pm = wp.tile([L, B], F32, tag="pm", name="pm")
                nc.vector.tensor_tensor(pm[:], g[:], psE[:, 0:B], op=OP.mult)
                nc.vector.tensor_tensor(ym[nxt][:], yode[:], pm[:], op=OP.subtract)

                xa = wp.tile([L, B], F32, tag="xa", name="xa")
                nc.scalar.activation(xa[:], psE[:, B:], AF.Abs,
                                     bias=w["ns2_bs"][:, 0:1])
                wx = wp.tile([L, B], F32, tag="wx", name="wx")
                nc.scalar.activation(wx[:], xa[:], AF.Exp, scale=-1.0)
                vx = wp.tile([L, B], F32, tag="vx", name="vx")
                nc.scalar.activation(vx[:], wx[:], AF.Exp, scale=-LN2)
                h0 = wp.tile([L, B], F32, tag="h0", name="h0")
                nc.vector.scalar_tensor_tensor(
                    h0[:], wx[:], LN2, rl[:], op0=OP.mult, op1=OP.add)
                h1 = wp.tile([L, B], F32, tag="h1", name="h1")
                nc.vector.scalar_tensor_tensor(
                    h1[:], h0[:], cc, ys[cur][:], op0=OP.add, op1=OP.subtract)
                aw = wp.tile([L, B], F32, tag="aw", name="aw")
                nc.vector.scalar_tensor_tensor(
                    aw[:], wx[:], 1.0, vx[:], op0=OP.add, op1=OP.mult)
                h2 = wp.tile([L, B], F32, tag="h2", name="h2")
                nc.vector.tensor_tensor(h2[:], h1[:], aw[:], op=OP.add)
                p1 = wp.tile([L, B], F32, tag="p1", name="p1")
                nc.vector.tensor_tensor(p1[:], g[:], h2[:], op=OP.mult)
                nc.vector.tensor_tensor(ys[nxt][:], ys[cur][:], p1[:],
                                        op=OP.subtract)

            # ---- final transform ----
            fin = n_tp % 2
            psB = pp.tile([128, 2 * B], F32, tag="psB", name="psB")
            for m in range(2):
                sl = psB[:, m * B:(m + 1) * B]
                ms = slice(m * 128, (m + 1) * 128)
                mm(sl, w["tz1_b"][:, ms], ones_row[:], start=True, stop=False)
                mm(sl, w["tz1_k0"][:, ms], ym[fin][:], start=False, stop=False)
                mm(sl, w["tz1_k1"][:, ms], ys[fin][:], start=False, stop=True)
            h_tz = wp.tile([128, 2 * B], B16, tag="h_ode", name="h_tz")
            nc.scalar.activation(h_tz[:], psB[:], AF.Tanh)
            psE = pp.tile([128, 2 * B], F32, tag="psE", name="psE2")
            for m in range(2):
                sl = psE[:, m * B:(m + 1) * B]
                ms = slice(m * 128, (m + 1) * 128)
                mm(sl, w["tz2_k0"][:, ms], h_tz[:, 0:B], start=True, stop=False)
                mm(sl, w["tz2_k1"][:, ms], h_tz[:, B:], start=False, stop=True)
            o_m = wp.tile([L, B], F32, tag="o_m", name="o_m")
            nc.scalar.activation(o_m[:], psE[:, 0:B], AF.Identity,
                                 bias=w["tz2_bm"][:, 0:1])
            o_s = wp.tile([L, B], F32, tag="o_s", name="o_s")
            nc.scalar.activation(o_s[:], psE[:, B:], AF.Abs,
                                 bias=w["tz2_bs"][:, 0:1])
            nc.sync.dma_start(d_om[:], o_m[:])
            nc.sync.dma_start(d_os[:], o_s[:])

    nc.compile()
    return nc


# --------------------------------------------------------------------------
# host-side packing
# --------------------------------------------------------------------------
def _dt_variants(obs, n_tp):
    F = np.float32
    dd = (obs[:-1] - obs[1:])[::-1]
    dts = np.concatenate([np.full((1,), -0.01, F), dd]).astype(F)
    uniq, vids = np.unique(dts, return_inverse=True)
    return uniq, tuple(int(v) for v in vids)


def _prep_in_maps(inputs, n_tp):
    F = np.float32
    d = {k: np.ascontiguousarray(np.asarray(v, F)) for k, v in inputs.items()}
    obs = d["obs_tps"][:n_tp]
    data = d["data"][:, :n_tp]

    uniq, vids = _dt_variants(obs, n_tp)

    # x slab: [t, c, subj] reversed in time; row 0 = 0.5 * (any-observed)
    xr = data.transpose(1, 2, 0)[::-1]                    # [t, 64, subj]
    m_row = F(0.5) * (xr[:, HALF:].sum(axis=1, keepdims=True) > 0)  # [t,1,subj]
    x_rev = np.concatenate([m_row, xr], axis=1).astype(BF)  # [t, 65, subj]
    x_rev = np.ascontiguousarray(x_rev)

    ns_w1s = d["ns_w1"].copy()
    ns_w1s[:2 * L] *= F(0.5)

    def kx(w1, b1):
        # row 0 = 0 (mask row), rows 1..64 = x weights, row 65 = bias
        return np.vstack([np.zeros((1, w1.shape[1]), F), w1[2 * L:], b1[None, :]])

    bf = {
        "ug1_k0": d["ug_w1"][:L], "ug1_k1": d["ug_w1"][L:2 * L],
        "ug1_kx": kx(d["ug_w1"], d["ug_b1"]),
        "rg1_k0": d["rg_w1"][:L], "rg1_k1": d["rg_w1"][L:2 * L],
        "rg1_kx": kx(d["rg_w1"], d["rg_b1"]),
        "ns1_k0": ns_w1s[:L], "ns1_k1": ns_w1s[L:2 * L],
        "ns1_kx": kx(d["ns_w1"], d["ns_b1"]),
        "ode1_w": d["ode_w1"],
        "ns2_k0": d["ns_w2"][:128], "ns2_k1": d["ns_w2"][128:],
        "ns2_bm16": d["ns_b2"][None, :L],
        "neg_eye": -np.eye(L, dtype=F),
        "tz1_k0": d["tz_w1"][:L], "tz1_k1": d["tz_w1"][L:],
        "tz1_b": d["tz_b1"][None, :],
        "tz2_k0": d["tz_w2"][:128], "tz2_k1": d["tz_w2"][128:],
    }
    for u, dtv in enumerate(uniq):
        bf[f"o2b_{u}"] = d["ode_b2"][None, :] * dtv
    shared = {k: np.ascontiguousarray(v.astype(BF)) for k, v in bf.items()}
    # fp8 DoubleRow stationaries: [part, ktile, M] with ktile = (rows 0:128,
    # rows 128:256) of the K=256 layer-2 weights
    F8 = ml_dtypes.float8_e4m3fn

    def k01(w2):
        return np.ascontiguousarray(
            np.stack([w2[:128], w2[128:]], axis=1).astype(F8))

    shared["ug2_k01"] = k01(d["ug_w2"])
    shared["rg2_k01"] = k01(d["rg_w2"])
    for u, dtv in enumerate(uniq):
        shared[f"o2k01_{u}"] = k01(d["ode_w2"] * dtv)
    shared["ode1_bc"] = np.ascontiguousarray(d["ode_b1"].reshape(2, 128).T)
    shared["ug2_bc"] = np.ascontiguousarray(d["ug_b2"][:, None] * F(0.5))
    shared["rg2_bc"] = np.ascontiguousarray(d["rg_b2"][:, None] * F(0.5))
    shared["ns2_bs"] = np.ascontiguousarray(d["ns_b2"][L:, None])
    shared["tz2_bm"] = np.ascontiguousarray(d["tz_b2"][:L, None])
    shared["tz2_bs"] = np.ascontiguousarray(d["tz_b2"][L:, None])

    in_maps = []
    for c in range(N_CORES):
        m = dict(shared)
        m["x_rev"] = np.ascontiguousarray(x_rev[:, :, c * B:(c + 1) * B])
        in_maps.append(m)
    return in_maps


def kernel(**inputs):
    from concourse.bass_utils import run_bass_kernel_spmd

    obs = np.asarray(inputs["obs_tps"], np.float32)[:N_TP]
    _, vids = _dt_variants(obs, N_TP)
    key = (N_TP, SP_ITERS, vids)
    if key not in _CACHE:
        _CACHE[key] = _build(N_TP, SP_ITERS, vids)
    nc = _CACHE[key]

    in_maps = _prep_in_maps(inputs, N_TP)
    res = run_bass_kernel_spmd(nc, in_maps, list(range(N_CORES)))
    outs = res.results

    mean = np.empty((1, N_SUBJ, L), np.float32)
    std = np.empty((1, N_SUBJ, L), np.float32)
    for c in range(N_CORES):
        mean[0, c * B:(c + 1) * B] = outs[c]["out_m"].T
        std[0, c * B:(c + 1) * B] = outs[c]["out_s"].T
    return mean, std
